# revision 79
# baseline (speedup 1.0000x reference)
"""Trainium2 Bass kernel for nn_Dnn_with_Attention (ragged attention-pooled DNN).

Contract: kernel(**inputs) takes FULL unsharded numpy inputs (keys as in
reference.setup_inputs()) and returns the FULL [256, 10] float32 output.

Strategy (data-parallel over utterances, 8 NeuronCores):
  - Host: balance the 256 segments over 8 cores (32 whole segments each,
    greedy LPT + swap refinement -> every core at exactly sum/8 frames on
    the benchmark lengths, so m_pad = 16384 = 32 chunks), gather each
    core's frames, pack x as fp8 feature-major pairs [64, 2, M_PAD] and
    build a per-frame one-hot segment membership matrix A.  A row of ones
    is appended as feature 78 so b1 folds into W1.  All four weight
    matrices are pre-quantized to fp8 e4m3 in DoubleRow pair layout.
  - Device (per core): ALL four layers run as fp8(e4m3) DoubleRow matmuls
    (two 128-row contraction tiles per pass); inter-layer activations are
    stored e4m3, h4 in bf16.  b4 is added via a tiny fp8 ones-row
    DoubleRow matmul inside the same PSUM accumulation group.
  - A fully static (no hardware loop) skewed software pipeline over
    512-frame chunks: at step s the PE runs L2(s-1) and L3(s-2) in 4-wide
    m-blocks with L1(s)'s m-pairs INTERLEAVED between them (so psA banks
    filled by L1 are never drained back-to-back by its own DVE-serial
    relus), then L4(s-3) in two halves with the (deferred, s-6)
    pooling matmuls between them, and scores(s-4).  x/A are
    DMA-prefetched per chunk 3 steps ahead; W1/W2 stream on the scalar
    engine's DMA queue at startup, in parallel with chunk 0's inputs.
  - Activations: L2/L3 m<6 + all L4 relus on scalar (bias via the
    activation unit), L1 relus + L2/L3 m>=6 on DVE; the score reduction
    is a DVE 2x tensor_tensor multiply + 4x tensor_scalar accumulate;
    exp on scalar; clamp/mask/eacc on gpsimd.  During the pipeline drain
    the last chunks alternate act engines per m-tile (latency, not
    throughput, binds there) and the last chunk's h4 requant runs as
    scalar Copy activations.
  - Segment softmax pooling as fp8 DoubleRow PE matmuls contracting TWO
    128-frame tiles per pass (a DR pair (k, k+128) is the SAME partition
    of two consecutive frame-tiles, so E and a gpsimd-requantized fp8
    copy of h4 are stored as pair tiles [128, 2, *] with no partition
    shuffle), accumulated into two persistent PSUM banks (DR matmuls
    require dst partition 0), deferred two steps behind the score chain.
    The softmax denominator is an SBUF f32r accumulator (eacc += E on
    gpsimd, the same quantized values the numerator uses) reduced by one
    tiny matmul in the tail.
  - Tail: the final per-utterance MLP runs once in f32r; the softmax
    normalization is folded into the W6 activation (per-partition
    scale = 1/denom; b6 enters pre-scaled by denom via a rank-1 matmul
    with the denominator row), and the pooled->pooledT transposes are
    software-pipelined with the W6 k-subtile matmuls.
"""

import sys

sys.path.insert(0, "/opt/trn_rl_repo")

import numpy as np
import ml_dtypes

import concourse.bass as bass
import concourse.mybir as mybir
import concourse.tile as tile
from concourse import bacc
from concourse.bass_utils import run_bass_kernel_spmd

P = 128
FEAT = 78
HID = 1024
NCLS = 10
NSEG = 256
NCORES = 8
SEGS_PER_CORE = NSEG // NCORES
CH = 512           # frames per chunk (free dim of layer-1..3 matmuls)
FRT_PER_CH = CH // P
NCOL = 256         # moving-dim columns per DoubleRow matmul (HW limit)
KS = HID // P      # 8 k-subtiles
KP = KS // 2       # 4 DoubleRow k-pairs
F32 = mybir.dt.float32
F32R = mybir.dt.float32r
F8 = mybir.dt.float8e4
BF16 = mybir.dt.bfloat16
E4NP = ml_dtypes.float8_e4m3

# misc constant tile column layout ([128, 256] f32, host-packed)
MC_B2 = 0          # cols 0..7   : b2 striped [128, 8]
MC_B3 = 8          # cols 8..15  : b3 striped
MC_B5 = 17         # col 17      : b5 replicated down partitions
MC_ID = 128        # cols 128..255: four 32x32 identity blocks, block q at
                   # rows 32q..32q+32, cols 128+32q..128+32q+32
# f32r matmul-constants tile ([128, 128])
MM_ONES = 0        # cols 0..7   : ones columns (denom matmul rhs, N=8)
MM_W7 = 16         # cols 16..95 : W7 as [128, 8, 10]
# row constants tile ([1, 192] f32r, host-packed)
RW_ONES = 0        # cols 0..127 : ones row
RW_B7 = 128        # cols 128..137 : b7


def _segment_ids(lengths: np.ndarray, total: int) -> np.ndarray:
    """Replicate jnp.repeat(arange(n), lengths, total_repeat_length=total)."""
    lengths = np.asarray(lengths, dtype=np.int64)
    seg = np.repeat(np.arange(lengths.shape[0], dtype=np.int32), np.maximum(lengths, 0))
    if seg.shape[0] >= total:
        return seg[:total]
    pad_val = seg[-1] if seg.shape[0] > 0 else np.int32(0)
    return np.concatenate([seg, np.full(total - seg.shape[0], pad_val, np.int32)])


def _balance_segments(lengths: np.ndarray) -> list[list[int]]:
    """Assign 256 segments to 8 cores, 32 each, minimizing max frame count.

    Greedy LPT init + single-move / pairwise-swap local search.  On the
    benchmark lengths this reaches a perfect partition (all cores at
    sum/8 frames), saving one 512-frame chunk of padding.
    """
    lengths = np.asarray(lengths, dtype=np.int64)
    cap = SEGS_PER_CORE
    order = np.argsort(-lengths, kind="stable")
    loads = [0] * NCORES
    bins: list[list[int]] = [[] for _ in range(NCORES)]
    for s in order:
        cands = [c for c in range(NCORES) if len(bins[c]) < cap]
        c = min(cands, key=lambda c: (loads[c], c))
        bins[c].append(int(s))
        loads[c] += int(lengths[s])
    target = (int(lengths.sum()) + NCORES - 1) // NCORES
    for _ in range(4096):
        hi = max(range(NCORES), key=lambda c: loads[c])
        if loads[hi] <= target:
            break
        best = None
        for other in range(NCORES):
            if other == hi:
                continue
            if len(bins[other]) < cap:
                for i, s in enumerate(bins[hi]):
                    d = int(lengths[s])
                    newmax = max(loads[hi] - d, loads[other] + d)
                    if best is None or newmax < best[0]:
                        best = (newmax, hi, other, i, None)
            for i, s1 in enumerate(bins[hi]):
                d1 = int(lengths[s1])
                for j, s2 in enumerate(bins[other]):
                    d2 = int(lengths[s2])
                    if d1 <= d2:
                        continue
                    newmax = max(loads[hi] - d1 + d2, loads[other] + d1 - d2)
                    if best is None or newmax < best[0]:
                        best = (newmax, hi, other, i, j)
        if best is None or best[0] >= loads[hi]:
            break
        _, a, b, i, j = best
        s1 = bins[a].pop(i)
        if j is None:
            bins[b].append(s1)
            loads[a] -= int(lengths[s1])
            loads[b] += int(lengths[s1])
        else:
            s2 = bins[b][j]
            bins[b][j] = s1
            bins[a].append(s2)
            loads[a] += int(lengths[s2]) - int(lengths[s1])
            loads[b] += int(lengths[s1]) - int(lengths[s2])
    for b in bins:
        b.sort()
    return bins


PRE = 3            # chunk DMA prefetch distance (steps)


def _build_program(m_pad: int):
    """Emit the fully static Bass/Tile program for one core (m_pad frames)."""
    nch = m_pad // CH
    frt = m_pad // P
    S = SEGS_PER_CORE

    nc = bacc.Bacc("TRN2", target_bir_lowering=False, debug=False,
                   num_devices=NCORES)

    xT_d = nc.dram_tensor("xq", [P // 2, 2, m_pad], F8, kind="ExternalInput")
    A_d = nc.dram_tensor("Amat", [P, frt, S], F32, kind="ExternalInput")
    W1_d = nc.dram_tensor("W1q", [P // 2, 2, HID], F8, kind="ExternalInput")
    W2_d = nc.dram_tensor("W2q", [KP, P, 2, HID], F8, kind="ExternalInput")
    W3_d = nc.dram_tensor("W3q", [KP, P, 2, HID], F8, kind="ExternalInput")
    W4_d = nc.dram_tensor("W4q", [KP, P, 2, HID], F8, kind="ExternalInput")
    W5_d = nc.dram_tensor("W5rep", [P, HID], BF16, kind="ExternalInput")
    W6_d = nc.dram_tensor("W6", [HID, HID], F32R, kind="ExternalInput")
    b4_d = nc.dram_tensor("b4q", [1, 2, HID], F8, kind="ExternalInput")
    on_d = nc.dram_tensor("onesq", [1, 2, P], F8, kind="ExternalInput")
    b6_d = nc.dram_tensor("b6r", [1, HID], F32R, kind="ExternalInput")
    misc_d = nc.dram_tensor("miscc", [P, 256], F32, kind="ExternalInput")
    mmc_d = nc.dram_tensor("mmcc", [P, P], F32R, kind="ExternalInput")
    row_d = nc.dram_tensor("rowm", [1, 192], F32R, kind="ExternalInput")
    out_d = nc.dram_tensor("out", [S, NCLS], F32, kind="ExternalOutput")

    RELU = mybir.ActivationFunctionType.Relu
    EXP = mybir.ActivationFunctionType.Exp
    MULT = mybir.AluOpType.mult
    ADD = mybir.AluOpType.add
    MAX = mybir.AluOpType.max
    DR = mybir.MatmulPerfMode.DoubleRow

    with tile.TileContext(nc) as tc:
        with (
            tc.tile_pool(name="wpool", bufs=1) as wpool,
            tc.tile_pool(name="xpool", bufs=4) as xpool,
            tc.tile_pool(name="apool", bufs=8) as apool,
            tc.tile_pool(name="hpool", bufs=2) as hpool,
            tc.tile_pool(name="h1pool", bufs=3) as h1pool,
            tc.tile_pool(name="h4pool", bufs=18) as h4pool,
            tc.tile_pool(name="spool", bufs=2) as spool,
            tc.tile_pool(name="colpool", bufs=8) as colpool,
            tc.tile_pool(name="epool", bufs=12) as epool,
            tc.tile_pool(name="psA", bufs=4, space="PSUM") as psA,
            tc.tile_pool(name="psB", bufs=2, space="PSUM") as psB,
            tc.tile_pool(name="psAcc", bufs=1, space="PSUM") as psAcc,
        ):
            # ---- chunk input prefetch (per-chunk x/A slices) ----
            xs: dict[int, object] = {}
            As: dict[int, object] = {}

            def prefetch(c):
                if not (0 <= c < nch) or c in xs:
                    return
                xt = xpool.tile([P // 2, 2, CH], F8, tag="x")
                nc.sync.dma_start(xt[:], xT_d.ap()[:, :, c * CH:(c + 1) * CH])
                at = apool.tile([P, FRT_PER_CH, S], F32, tag="A")
                nc.sync.dma_start(
                    at[:], A_d.ap()[:, c * FRT_PER_CH:(c + 1) * FRT_PER_CH, :])
                xs[c] = xt
                As[c] = at

            # ---- resident constants/weights.  DMA issue order tracks first
            # use: W1 + chunk 0/1 first, then each layer's weights in the
            # order the pipeline-fill steps consume them; tail-only
            # constants (mmc/rowm/b6/W6) last.
            W1s = wpool.tile([P // 2, 2, HID], F8, tag="W1")
            # startup: W1 on the (idle) scalar queue so it runs in
            # parallel with chunk 0/1's x/A DMAs on the SP queue
            nc.gpsimd.dma_start(W1s[:], W1_d.ap())
            prefetch(0)
            prefetch(1)
            W2q = [wpool.tile([P, 2, HID], F8, tag=f"W2q{j}", name=f"W2q{j}")
                   for j in range(KP)]
            W3q = [wpool.tile([P, 2, HID], F8, tag=f"W3q{j}", name=f"W3q{j}")
                   for j in range(KP)]
            W4q = [wpool.tile([P, 2, HID], F8, tag=f"W4q{j}", name=f"W4q{j}")
                   for j in range(KP)]
            for j in range(KP):
                nc.scalar.dma_start(W2q[j][:], W2_d.ap()[j])
            misc = wpool.tile([P, 256], F32, tag="misc")
            nc.sync.dma_start(misc[:], misc_d.ap())
            prefetch(2)
            for j in range(KP):
                nc.sync.dma_start(W3q[j][:], W3_d.ap()[j])
            b4qs = wpool.tile([1, 2, HID], F8, tag="b4q")
            nc.sync.dma_start(b4qs[:], b4_d.ap())
            onesq = wpool.tile([1, 2, P], F8, tag="onesq")
            nc.sync.dma_start(onesq[:], on_d.ap())
            for j in range(KP):
                nc.sync.dma_start(W4q[j][:], W4_d.ap()[j])
            W5s = wpool.tile([P, HID], BF16, tag="W5")
            nc.sync.dma_start(W5s[:], W5_d.ap())
            mmc = wpool.tile([P, P], F32R, tag="mmc")
            nc.sync.dma_start(mmc[:], mmc_d.ap())
            rowm = wpool.tile([1, 192], F32R, tag="rowm")
            nc.sync.dma_start(rowm[:], row_d.ap())
            b6s = wpool.tile([1, HID], F32R, tag="b6")
            nc.sync.dma_start(b6s[:], b6_d.ap())
            W6s = []
            for k in range(KS):
                t = wpool.tile([P, HID], F32R, tag=f"W6k{k}")
                nc.sync.dma_start(t[:], W6_d.ap()[k * P:(k + 1) * P, :])
                W6s.append(t)

            ones_col = mmc[:, MM_ONES:MM_ONES + 8]
            b5col = misc[:, MC_B5:MC_B5 + 1]
            idents = [misc[32 * q:32 * q + 32, MC_ID + 32 * q:MC_ID + 32 * q + 32]
                      for q in range(4)]
            W7v = mmc[:, MM_W7:MM_W7 + KS * NCLS].rearrange(
                "p (o c) -> p o c", c=NCLS)
            b7row = rowm[:, RW_B7:RW_B7 + NCLS]
            ones_row = rowm[:, RW_ONES:RW_ONES + P]

            # persistent PSUM pooled accumulators, one bank per hidden
            # half at partitions 0..31 (DoubleRow matmuls require dst
            # partition 0)
            pooled0 = psAcc.tile([S, 512], F32, tag="pooled0")
            pooled1 = psAcc.tile([S, 512], F32, tag="pooled1")
            # softmax denominator accumulator (f32r: feeds the tail matmul)
            eacc = wpool.tile([P, S], F32R, tag="eacc")

            # ---- per-chunk layer emitters (skewed pipeline below) ----
            h1s: dict[int, object] = {}
            h2s: dict[int, object] = {}
            h3s: dict[int, object] = {}
            h4s: dict[int, list] = {}
            pend: list[tuple] = []

            def do_L1(u, n_scalar_relu=0, ms=None, h1_cur=None):
                # fp8 DoubleRow over two 64-feature halves (b1 folded via
                # ones feature); relu on DVE (scalar split during pipeline
                # fill when the scalar engine is otherwise idle).  h1 is
                # stored as four per-DR-pair tiles [P, 2, CH] so each of
                # L2's j-matmuls depends only on the two relu slices it
                # actually reads.
                if ms is None:
                    ms = range(KS)
                xg = xs[u]
                if h1_cur is None:
                    h1_cur = [h1pool.tile([P, 2, CH], F8, tag=f"h1p{j}",
                                          name=f"h1p{j}") for j in range(KP)]
                h1 = h1_cur
                for m in ms:
                    ps = psA.tile([P, CH], F32, tag="mm")
                    for c2 in range(CH // NCOL):
                        xt = xg[:, :, c2 * NCOL:(c2 + 1) * NCOL]
                        nc.tensor.matmul(ps[:, c2 * NCOL:(c2 + 1) * NCOL],
                                         W1s[:, :, m * P:(m + 1) * P],
                                         xt, start=True, stop=True,
                                         perf_mode=DR)
                    if m < n_scalar_relu:
                        nc.scalar.activation(h1[m // 2][:, m % 2, :], ps[:],
                                             RELU)
                    else:
                        nc.vector.tensor_scalar_max(h1[m // 2][:, m % 2, :],
                                                    ps[:], 0.0)
                h1s[u] = h1
                if max(ms) == KS - 1:
                    xs.pop(u)
                return h1

            def do_L23(u, Wq, boff, li, ms=None, h_cur=None, alt=False):
                # fp8 DoubleRow; relu+bias on scalar (m<6) / DVE (m>=6)
                if ms is None:
                    ms = range(KS)
                h_in = h1s[u] if li == 2 else h2s[u]
                tag = "h2p" if li == 2 else "h3p"
                if h_cur is None:
                    h_cur = [hpool.tile([P, 2, CH], F8, tag=f"{tag}{j}",
                                        name=f"{tag}{j}") for j in range(KP)]
                h_out = h_cur
                for m in ms:
                    ps = psA.tile([P, CH], F32, tag="mm")
                    for c2 in range(CH // NCOL):
                        seg = ps[:, c2 * NCOL:(c2 + 1) * NCOL]
                        for j in range(KP):
                            nc.tensor.matmul(
                                seg, Wq[j][:, :, m * P:(m + 1) * P],
                                h_in[j][:, :, c2 * NCOL:(c2 + 1) * NCOL],
                                start=(j == 0), stop=(j == KP - 1),
                                perf_mode=DR)
                    bcol = misc[:, boff + m:boff + m + 1]
                    if (m % 2 == 1) if alt else (m >= 6):
                        nc.vector.tensor_scalar(
                            out=h_out[m // 2][:, m % 2, :], in0=ps[:],
                            scalar1=bcol, scalar2=0.0,
                            op0=ADD, op1=MAX)
                    else:
                        nc.scalar.activation(
                            h_out[m // 2][:, m % 2, :], ps[:], RELU,
                            bias=bcol)
                (h2s if li == 2 else h3s)[u] = h_out
                if max(ms) == KS - 1:
                    (h1s if li == 2 else h2s).pop(u)
                return h_out

            def do_L4(u, alt=False, fs=None):
                # frame-major fp8 DR; relu on scalar; h4 bf16 (score DVE 2x
                # rate + 1 cycle/row pooling matmuls)
                if fs is None:
                    fs = range(FRT_PER_CH)
                h3 = h3s[u]
                tiles = h4s.get(u, [])
                for f in fs:
                    h4 = h4pool.tile([P, HID], BF16, tag="h4")
                    for n in range(2):
                        ps4 = psB.tile([P, 512], F32, tag="l4")
                        for c2 in range(2):
                            seg = ps4[:, c2 * 256:(c2 + 1) * 256]
                            col0 = n * 512 + c2 * 256
                            for j in range(KP):
                                nc.tensor.matmul(
                                    seg,
                                    h3[j][:, :, f * P:(f + 1) * P],
                                    W4q[j][:, :, col0:col0 + 256],
                                    start=(j == 0), stop=False,
                                    perf_mode=DR)
                            nc.tensor.matmul(
                                seg, onesq[:],
                                b4qs[:, :, col0:col0 + 256],
                                start=False, stop=True, perf_mode=DR)
                        if alt and n == 1:
                            nc.vector.tensor_scalar_max(
                                h4[:, n * 512:(n + 1) * 512], ps4[:], 0.0)
                        else:
                            nc.scalar.activation(
                                h4[:, n * 512:(n + 1) * 512], ps4[:], RELU)
                    tiles.append(h4)
                h4s[u] = tiles
                if max(fs) == FRT_PER_CH - 1:
                    h3s.pop(u)

            def do_scores(u, step):
                # d = sum(h4*W5rep) via DVE mult (2x) + reduce (4x);
                # e = max(exp(d + b5), 1) on scalar/gpsimd; E = A*e and
                # eacc += E on gpsimd.  For the fp8 DoubleRow pooling, h4
                # is also requantized to fp8 PAIR tiles [128, 2, 1024]
                # (slot i = frame-tile 2p+i) on gpsimd — emitted FIRST so
                # the (long-ready) copies fill the Pool queue's idle front
                # while the et chain waits for the exp results.
                ag = As.pop(u)
                first_u = (u == 0)
                tiles = h4s.pop(u)
                h4q = []
                CPY = mybir.ActivationFunctionType.Copy
                for pi in range(FRT_PER_CH // 2):
                    hq = h4pool.tile([P, 2, HID], F8, tag="h4q")
                    if u >= nch - 1:
                        # drain: scalar engine is idle; freeing the Pool
                        # queue lets the final et chain -> pooling finish
                        # sooner
                        nc.scalar.activation(hq[:, 0, :], tiles[2 * pi][:],
                                             CPY)
                        nc.scalar.activation(hq[:, 1, :],
                                             tiles[2 * pi + 1][:], CPY)
                    else:
                        nc.gpsimd.tensor_scalar_add(hq[:, 0, :],
                                                    tiles[2 * pi][:], 0.0)
                        nc.gpsimd.tensor_scalar_add(hq[:, 1, :],
                                                    tiles[2 * pi + 1][:], 0.0)
                    h4q.append(hq)
                etp = None
                for f, h4 in enumerate(tiles):
                    prod = spool.tile([P, HID], BF16, tag="sc")
                    ct = colpool.tile([P, 16], F32, tag="col")
                    nc.vector.tensor_tensor(
                        out=prod[:], in0=h4[:], in1=W5s[:], op=MULT)
                    nc.vector.tensor_scalar(
                        out=prod[:], in0=prod[:], scalar1=1.0,
                        scalar2=0.0, op0=MULT, op1=ADD,
                        accum_out=ct[:, 0:1])
                    nc.scalar.activation(ct[:, 1:2], ct[:, 0:1], EXP,
                                         bias=b5col)
                    nc.gpsimd.tensor_scalar_max(ct[:, 2:3], ct[:, 1:2], 1.0)
                    if f % 2 == 0:
                        etp = epool.tile([P, 2, S], F8, tag="E")
                    nc.gpsimd.tensor_scalar_mul(
                        etp[:, f % 2, :], ag[:, f, :], ct[:, 2:3])
                    # eacc accumulates the SAME fp8-quantized E values the
                    # pooling numerator uses, so softmax weights sum to 1
                    if first_u and f == 0:
                        nc.gpsimd.tensor_scalar_add(eacc[:],
                                                    etp[:, 0, :], 0.0)
                    else:
                        nc.gpsimd.tensor_tensor(
                            out=eacc[:], in0=eacc.bitcast(F32)[:],
                            in1=etp[:, f % 2, :], op=ADD)
                    if f % 2 == 1:
                        st = bool(first_u and f == 1)
                        sp = bool(u == nch - 1 and f == FRT_PER_CH - 1)
                        pend.append((etp, h4q[f // 2], st, sp, step))

            def flush_pool(max_step, limit=1 << 30):
                # fp8 DoubleRow pooling matmuls (K=256: two frame-tiles per
                # pass) for score chains born at step <= max_step: 4
                # col-group quarter matmuls into the single pooled bank
                while pend and pend[0][4] <= max_step and limit > 0:
                    limit -= 1
                    etp, hq, st, sp, _ = pend.pop(0)
                    for h, pl in enumerate((pooled0, pooled1)):
                        # one full-row (N=512) fp8 DoubleRow matmul per
                        # hidden half (contraction = two frame-tiles)
                        nc.tensor.matmul(
                            pl[:], etp[:],
                            hq[:, :, 512 * h:512 * (h + 1)],
                            start=st, stop=sp, perf_mode=DR)

            # ---- main static pipeline over chunks ----
            # 4-deep skewed pipeline; chunk 0's L1 is pre-emitted with its
            # relus split across the still-idle scalar engine so step 1's
            # L2(0) never races the DVE queue.
            if nch >= 1:
                do_L1(0, n_scalar_relu=4)
            prefetch(3)
            for s in range(1, nch + 5):
                # interleave L1's m-pairs between the L2/L3 m-blocks so
                # psA banks filled by L1 are never drained back-to-back by
                # its own (DVE-serial) relus
                l1 = s < nch
                h1c = h2c = h3c = None
                if l1:
                    prefetch(s + PRE)
                if l1:
                    h1c = do_L1(s, ms=range(0, 2))
                if 0 <= s - 1 < nch:
                    h2c = do_L23(s - 1, W2q, MC_B2, 2, ms=range(0, 4),
                                 alt=(s - 1 >= nch - 2))
                if l1:
                    do_L1(s, ms=range(2, 4), h1_cur=h1c)
                if 0 <= s - 1 < nch:
                    do_L23(s - 1, W2q, MC_B2, 2, ms=range(4, 8), h_cur=h2c,
                           alt=(s - 1 >= nch - 2))
                if l1:
                    do_L1(s, ms=range(4, 6), h1_cur=h1c)
                if 0 <= s - 2 < nch:
                    h3c = do_L23(s - 2, W3q, MC_B3, 3, ms=range(0, 4),
                                 alt=(s - 2 >= nch - 2))
                if l1:
                    do_L1(s, ms=range(6, 8), h1_cur=h1c)
                if 0 <= s - 2 < nch:
                    do_L23(s - 2, W3q, MC_B3, 3, ms=range(4, 8), h_cur=h3c,
                           alt=(s - 2 >= nch - 2))
                if 0 <= s - 3 < nch:
                    do_L4(s - 3, alt=(s - 3 >= nch - 2), fs=(0, 1))
                    flush_pool(s - 2)
                    do_L4(s - 3, alt=(s - 3 >= nch - 2), fs=(2, 3))
                else:
                    flush_pool(s - 2)
                if 0 <= s - 4 < nch:
                    do_scores(s - 4, s)
                if s >= nch:
                    # pipeline drain: no L1..L4 work left to hide behind,
                    # flush pooling as soon as the score chain lands
                    flush_pool(s)
            flush_pool(1 << 30)

            # ---- tail: final per-utterance MLP (f32r) ----
            # denom[s] = sum_p eacc[p, s] via one tiny PE matmul -> [S, 1]
            psd = psA.tile([S, 8], F32, tag="mm")
            nc.tensor.matmul(psd[:], eacc[:], ones_col,
                             start=True, stop=True)
            fc = colpool.tile([P, 16], F32, tag="col")
            nc.vector.tensor_copy(out=fc[:S, 0:1], in_=psd[:, 0:1])
            nc.vector.reciprocal(fc[:S, 1:2], fc[:S, 0:1])
            # denom as an f32r row [1, S] (rank-1 b6*denom matmul lhsT)
            psr = psA.tile([1, S], F32, tag="mm")
            nc.tensor.transpose(psr[:], fc[:S, 0:1], idents[0])
            drow = wpool.tile([1, S], F32R, tag="drow")
            nc.vector.tensor_copy(out=drow[:], in_=psr[:])

            # pooled PSUM -> SBUF (unscaled; normalization is folded
            # into the W6 activation below)
            pooled_sb = wpool.tile([S, HID], F32, tag="poolsb")
            nc.vector.tensor_copy(out=pooled_sb[:, :512], in_=pooled0[:])
            nc.vector.tensor_copy(out=pooled_sb[:, 512:], in_=pooled1[:])

            # transpose pooled -> pooledT [hid, seg], software-pipelined with
            # the W6 matmuls: psg(n) accumulates k-subtile products as soon
            # as pooledT[:, k] lands
            tposed = wpool.tile([P, KS, 2 * S], F32R, tag="tposed")
            pooledT = tposed[:, :, :S]
            gT = tposed[:, :, S:]
            psg0 = psB.tile([S, 512], F32, tag="l4")
            psg1 = psB.tile([S, 512], F32, tag="l4")
            for k in range(KS):
                pst = psA.tile([P, S], F32, tag="mm")
                nc.tensor.transpose(
                    pst[:], pooled_sb[:, 128 * k:128 * (k + 1)], idents[0])
                nc.vector.tensor_copy(out=pooledT[:, k, :], in_=pst[:])
                nc.tensor.matmul(psg0[:], pooledT[:, k, :],
                                 W6s[k][:, :512],
                                 start=(k == 0), stop=False)
                nc.tensor.matmul(psg1[:], pooledT[:, k, :],
                                 W6s[k][:, 512:],
                                 start=(k == 0), stop=False)

            # g = relu((pooled @ W6 + denom*b6) / denom)  (seg-major [S, HID])
            # drow = denom row: psg accumulates pooled@W6 + denom*b6
            g_sb = spool.tile([S, HID], F32, tag="sc")
            for n, psg in enumerate((psg0, psg1)):
                nc.tensor.matmul(psg[:], drow[:],
                                 b6s[:, n * 512:(n + 1) * 512],
                                 start=False, stop=True)
                nc.scalar.activation(g_sb[:, n * 512:(n + 1) * 512], psg[:],
                                     RELU, scale=fc[:S, 1:2])

            # gT [hid, seg]
            for k in range(KS):
                pst = psA.tile([P, S], F32, tag="mm")
                nc.tensor.transpose(pst[:], g_sb[:, k * P:(k + 1) * P],
                                    idents[0])
                nc.vector.tensor_copy(out=gT[:, k, :], in_=pst[:])

            # out = g @ W7 + b7
            pso = psA.tile([S, NCLS], F32, tag="mm")
            for k in range(KS):
                nc.tensor.matmul(pso[:], gT[:, k, :], W7v[:, k, :],
                                 start=(k == 0), stop=False)
            nc.tensor.matmul(pso[:], ones_row[:, :S], b7row,
                             start=False, stop=True)
            oc = colpool.tile([S, 16], F32, tag="col")
            nc.vector.tensor_copy(out=oc[:, :NCLS], in_=pso[:])
            nc.sync.dma_start(out_d.ap()[:], oc[:, :NCLS])

    nc.compile()
    return nc


def _q8(a: np.ndarray) -> np.ndarray:
    return np.asarray(a, dtype=np.float32).astype(E4NP)


def _pack_dr(W: np.ndarray) -> np.ndarray:
    """[1024, N] weight matrix -> DoubleRow fp8 layout [KP, 128, 2, N]."""
    return np.ascontiguousarray(
        _q8(W).reshape(KP, 2, P, -1).transpose(0, 2, 1, 3))


def prepare_inputs(x, W1, b1, W2, b2, W3, b3, W4, b4, W5, b5, W6, b6, W7, b7,
                   lengths):
    """Host-side sharding/packing. Returns (in_maps, bins, m_pad)."""
    x = np.ascontiguousarray(np.asarray(x, dtype=np.float32))
    lengths = np.asarray(lengths)
    total = x.shape[0]
    seg_ids = _segment_ids(lengths, total)
    counts = np.bincount(seg_ids, minlength=NSEG).astype(np.int64)
    starts = np.zeros(NSEG + 1, dtype=np.int64)
    starts[1:] = np.cumsum(counts)

    bins = _balance_segments(counts)
    core_frames = [int(sum(counts[s] for s in b)) for b in bins]
    m_pad = ((max(core_frames) + CH - 1) // CH) * CH
    frt = m_pad // P

    W1p = np.zeros((P, HID), dtype=np.float32)
    W1p[:FEAT] = np.asarray(W1, dtype=np.float32)
    W1p[FEAT] = np.asarray(b1, dtype=np.float32)
    # DoubleRow over two 64-feature halves: [64, 2, HID]
    W1q = np.ascontiguousarray(
        _q8(W1p).reshape(2, P // 2, HID).transpose(1, 0, 2))

    misc = np.zeros((P, 256), dtype=np.float32)
    misc[:, MC_B2:MC_B2 + KS] = np.asarray(b2, np.float32).reshape(KS, P).T
    misc[:, MC_B3:MC_B3 + KS] = np.asarray(b3, np.float32).reshape(KS, P).T
    misc[:, MC_B5] = np.float32(np.asarray(b5, np.float32).reshape(-1)[0])
    for q in range(4):
        misc[32 * q:32 * q + 32, MC_ID + 32 * q:MC_ID + 32 * q + 32] = np.eye(
            32, dtype=np.float32)

    mmcc = np.zeros((P, P), dtype=np.float32)
    mmcc[:, MM_ONES:MM_ONES + 8] = 1.0
    mmcc[:, MM_W7:MM_W7 + KS * NCLS] = np.asarray(W7, np.float32).reshape(
        KS, P, NCLS).transpose(1, 0, 2).reshape(P, KS * NCLS)

    rowm = np.zeros((1, 192), dtype=np.float32)
    rowm[0, RW_ONES:RW_ONES + P] = 1.0
    rowm[0, RW_B7:RW_B7 + NCLS] = np.asarray(b7, np.float32).reshape(-1)

    b4q = np.zeros((1, 2, HID), dtype=E4NP)
    b4q[0, 0, :] = _q8(np.asarray(b4, np.float32).reshape(-1))
    onesq = np.zeros((1, 2, P), dtype=E4NP)
    onesq[0, 0, :] = np.float32(1.0)

    shared = dict(
        W1q=W1q,
        W2q=_pack_dr(np.asarray(W2, np.float32)),
        W3q=_pack_dr(np.asarray(W3, np.float32)),
        W4q=_pack_dr(np.asarray(W4, np.float32)),
        W5rep=np.broadcast_to(
            np.asarray(W5, np.float32).reshape(1, HID).astype(
                ml_dtypes.bfloat16), (P, HID)).copy(),
        W6=np.ascontiguousarray(np.asarray(W6, np.float32)),
        b4q=b4q,
        onesq=onesq,
        b6r=np.asarray(b6, np.float32).reshape(1, HID),
        miscc=misc,
        mmcc=mmcc,
        rowm=rowm,
    )

    in_maps = []
    for core in range(NCORES):
        segs = bins[core]
        xs = [x[starts[s]:starts[s + 1]] for s in segs]
        xcat = np.concatenate(xs, axis=0) if xs else np.zeros((0, FEAT), np.float32)
        n = xcat.shape[0]
        xT = np.zeros((P, m_pad), dtype=np.float32)
        xT[:FEAT, :n] = xcat.T
        xT[FEAT, :n] = 1.0  # constant feature -> b1
        A = np.zeros((m_pad, SEGS_PER_CORE), dtype=np.float32)
        off = 0
        for j, s in enumerate(segs):
            ln = int(counts[s])
            A[off:off + ln, j] = 1.0
            off += ln
        im = dict(shared)
        # fp8 x, DoubleRow halves: xq[p, i, col] = xpad[i*64 + p, col]
        im["xq"] = np.ascontiguousarray(
            _q8(xT).reshape(2, P // 2, m_pad).transpose(1, 0, 2))
        # partition-major layout [P, frt, S]: Ah[p, t, s] = A[t*128 + p, s]
        im["Amat"] = np.ascontiguousarray(
            A.reshape(frt, P, SEGS_PER_CORE).transpose(1, 0, 2))
        in_maps.append(im)
    return in_maps, bins, m_pad


_PROGRAM_CACHE: dict[int, object] = {}


def kernel(**inputs) -> np.ndarray:
    in_maps, bins, m_pad = prepare_inputs(**inputs)
    nc = _PROGRAM_CACHE.get(m_pad)
    if nc is None:
        nc = _build_program(m_pad)
        _PROGRAM_CACHE[m_pad] = nc
    res = run_bass_kernel_spmd(nc, in_maps, core_ids=list(range(NCORES)))
    out = np.zeros((NSEG, NCLS), dtype=np.float32)
    for core in range(NCORES):
        out[bins[core]] = res.results[core]["out"]
    return out


# revision 82
# speedup vs baseline: 1.0004x; 1.0004x over previous
"""Trainium2 Bass kernel for nn_Dnn_with_Attention (ragged attention-pooled DNN).

Contract: kernel(**inputs) takes FULL unsharded numpy inputs (keys as in
reference.setup_inputs()) and returns the FULL [256, 10] float32 output.

Strategy (data-parallel over utterances, 8 NeuronCores):
  - Host: balance the 256 segments over 8 cores (32 whole segments each,
    greedy LPT + swap refinement -> every core at exactly sum/8 frames on
    the benchmark lengths, so m_pad = 16384 = 32 chunks), gather each
    core's frames, pack x as fp8 feature-major pairs [64, 2, M_PAD] and
    build a per-frame one-hot segment membership matrix A.  A row of ones
    is appended as feature 78 so b1 folds into W1.  All four weight
    matrices are pre-quantized to fp8 e4m3 in DoubleRow pair layout.
  - Device (per core): ALL four layers run as fp8(e4m3) DoubleRow matmuls
    (two 128-row contraction tiles per pass); inter-layer activations are
    stored e4m3, h4 in bf16.  b4 is added via a tiny fp8 ones-row
    DoubleRow matmul inside the same PSUM accumulation group.
  - A fully static (no hardware loop) skewed software pipeline over
    512-frame chunks: at step s the PE runs L2(s-1) and L3(s-2) in 4-wide
    m-blocks with L1(s)'s m-pairs INTERLEAVED between them (so psA banks
    filled by L1 are never drained back-to-back by its own DVE-serial
    relus), then L4(s-3) in two halves with the (deferred, s-6)
    pooling matmuls between them, and scores(s-4).  x/A are
    DMA-prefetched per chunk 3 steps ahead; W1/W2 stream on the scalar
    engine's DMA queue at startup, in parallel with chunk 0's inputs.
  - Activations: L2/L3 m<6 + all L4 relus on scalar (bias via the
    activation unit), L1 relus + L2/L3 m>=6 on DVE; the score reduction
    is a DVE 2x tensor_tensor multiply + 4x tensor_scalar accumulate;
    exp on scalar; clamp/mask/eacc on gpsimd.  During the pipeline drain
    the last chunks alternate act engines per m-tile (latency, not
    throughput, binds there) and the last chunk's h4 requant runs as
    scalar Copy activations.
  - Segment softmax pooling as fp8 DoubleRow PE matmuls contracting TWO
    128-frame tiles per pass (a DR pair (k, k+128) is the SAME partition
    of two consecutive frame-tiles, so E and a gpsimd-requantized fp8
    copy of h4 are stored as pair tiles [128, 2, *] with no partition
    shuffle), accumulated into two persistent PSUM banks (DR matmuls
    require dst partition 0), deferred two steps behind the score chain.
    The softmax denominator is an SBUF f32r accumulator (eacc += E on
    gpsimd, the same quantized values the numerator uses) reduced by one
    tiny matmul in the tail.
  - Tail: the final per-utterance MLP runs once in f32r; the softmax
    normalization is folded into the W6 activation (per-partition
    scale = 1/denom; b6 enters pre-scaled by denom via a rank-1 matmul
    with the denominator row), and the pooled->pooledT transposes are
    software-pipelined with the W6 k-subtile matmuls.
"""

import sys

sys.path.insert(0, "/opt/trn_rl_repo")

import numpy as np
import ml_dtypes

import concourse.bass as bass
import concourse.mybir as mybir
import concourse.tile as tile
from concourse import bacc
from concourse.bass_utils import run_bass_kernel_spmd

P = 128
FEAT = 78
HID = 1024
NCLS = 10
NSEG = 256
NCORES = 8
SEGS_PER_CORE = NSEG // NCORES
CH = 512           # frames per chunk (free dim of layer-1..3 matmuls)
FRT_PER_CH = CH // P
NCOL = 256         # moving-dim columns per DoubleRow matmul (HW limit)
KS = HID // P      # 8 k-subtiles
KP = KS // 2       # 4 DoubleRow k-pairs
F32 = mybir.dt.float32
F32R = mybir.dt.float32r
F8 = mybir.dt.float8e4
BF16 = mybir.dt.bfloat16
E4NP = ml_dtypes.float8_e4m3

# misc constant tile column layout ([128, 256] f32, host-packed)
MC_B2 = 0          # cols 0..7   : b2 striped [128, 8]
MC_B3 = 8          # cols 8..15  : b3 striped
MC_B5 = 17         # col 17      : b5 replicated down partitions
MC_ID = 128        # cols 128..255: four 32x32 identity blocks, block q at
                   # rows 32q..32q+32, cols 128+32q..128+32q+32
# f32r matmul-constants tile ([128, 128])
MM_ONES = 0        # cols 0..7   : ones columns (denom matmul rhs, N=8)
MM_W7 = 16         # cols 16..95 : W7 as [128, 8, 10]
# row constants tile ([1, 192] f32r, host-packed)
RW_ONES = 0        # cols 0..127 : ones row
RW_B7 = 128        # cols 128..137 : b7


def _segment_ids(lengths: np.ndarray, total: int) -> np.ndarray:
    """Replicate jnp.repeat(arange(n), lengths, total_repeat_length=total)."""
    lengths = np.asarray(lengths, dtype=np.int64)
    seg = np.repeat(np.arange(lengths.shape[0], dtype=np.int32), np.maximum(lengths, 0))
    if seg.shape[0] >= total:
        return seg[:total]
    pad_val = seg[-1] if seg.shape[0] > 0 else np.int32(0)
    return np.concatenate([seg, np.full(total - seg.shape[0], pad_val, np.int32)])


def _balance_segments(lengths: np.ndarray) -> list[list[int]]:
    """Assign 256 segments to 8 cores, 32 each, minimizing max frame count.

    Greedy LPT init + single-move / pairwise-swap local search.  On the
    benchmark lengths this reaches a perfect partition (all cores at
    sum/8 frames), saving one 512-frame chunk of padding.
    """
    lengths = np.asarray(lengths, dtype=np.int64)
    cap = SEGS_PER_CORE
    order = np.argsort(-lengths, kind="stable")
    loads = [0] * NCORES
    bins: list[list[int]] = [[] for _ in range(NCORES)]
    for s in order:
        cands = [c for c in range(NCORES) if len(bins[c]) < cap]
        c = min(cands, key=lambda c: (loads[c], c))
        bins[c].append(int(s))
        loads[c] += int(lengths[s])
    target = (int(lengths.sum()) + NCORES - 1) // NCORES
    for _ in range(4096):
        hi = max(range(NCORES), key=lambda c: loads[c])
        if loads[hi] <= target:
            break
        best = None
        for other in range(NCORES):
            if other == hi:
                continue
            if len(bins[other]) < cap:
                for i, s in enumerate(bins[hi]):
                    d = int(lengths[s])
                    newmax = max(loads[hi] - d, loads[other] + d)
                    if best is None or newmax < best[0]:
                        best = (newmax, hi, other, i, None)
            for i, s1 in enumerate(bins[hi]):
                d1 = int(lengths[s1])
                for j, s2 in enumerate(bins[other]):
                    d2 = int(lengths[s2])
                    if d1 <= d2:
                        continue
                    newmax = max(loads[hi] - d1 + d2, loads[other] + d1 - d2)
                    if best is None or newmax < best[0]:
                        best = (newmax, hi, other, i, j)
        if best is None or best[0] >= loads[hi]:
            break
        _, a, b, i, j = best
        s1 = bins[a].pop(i)
        if j is None:
            bins[b].append(s1)
            loads[a] -= int(lengths[s1])
            loads[b] += int(lengths[s1])
        else:
            s2 = bins[b][j]
            bins[b][j] = s1
            bins[a].append(s2)
            loads[a] += int(lengths[s2]) - int(lengths[s1])
            loads[b] += int(lengths[s1]) - int(lengths[s2])
    for b in bins:
        b.sort()
    return bins


PRE = 3            # chunk DMA prefetch distance (steps)


def _build_program(m_pad: int):
    """Emit the fully static Bass/Tile program for one core (m_pad frames)."""
    nch = m_pad // CH
    frt = m_pad // P
    S = SEGS_PER_CORE

    nc = bacc.Bacc("TRN2", target_bir_lowering=False, debug=False,
                   num_devices=NCORES)

    xT_d = nc.dram_tensor("xq", [P // 2, 2, m_pad], F8, kind="ExternalInput")
    A_d = nc.dram_tensor("Amat", [P, frt, S], F32, kind="ExternalInput")
    W1_d = nc.dram_tensor("W1q", [P // 2, 2, HID], F8, kind="ExternalInput")
    W2_d = nc.dram_tensor("W2q", [KP, P, 2, HID], F8, kind="ExternalInput")
    W3_d = nc.dram_tensor("W3q", [KP, P, 2, HID], F8, kind="ExternalInput")
    W4_d = nc.dram_tensor("W4q", [KP, P, 2, HID], F8, kind="ExternalInput")
    W5_d = nc.dram_tensor("W5rep", [P, HID], BF16, kind="ExternalInput")
    W6_d = nc.dram_tensor("W6", [HID, HID], F32R, kind="ExternalInput")
    b4_d = nc.dram_tensor("b4q", [1, 2, HID], F8, kind="ExternalInput")
    on_d = nc.dram_tensor("onesq", [1, 2, P], F8, kind="ExternalInput")
    b6_d = nc.dram_tensor("b6r", [1, HID], F32R, kind="ExternalInput")
    misc_d = nc.dram_tensor("miscc", [P, 256], F32, kind="ExternalInput")
    mmc_d = nc.dram_tensor("mmcc", [P, P], F32R, kind="ExternalInput")
    row_d = nc.dram_tensor("rowm", [1, 192], F32R, kind="ExternalInput")
    out_d = nc.dram_tensor("out", [S, NCLS], F32, kind="ExternalOutput")

    RELU = mybir.ActivationFunctionType.Relu
    EXP = mybir.ActivationFunctionType.Exp
    MULT = mybir.AluOpType.mult
    ADD = mybir.AluOpType.add
    MAX = mybir.AluOpType.max
    DR = mybir.MatmulPerfMode.DoubleRow

    with tile.TileContext(nc) as tc:
        with (
            tc.tile_pool(name="wpool", bufs=1) as wpool,
            tc.tile_pool(name="xpool", bufs=4) as xpool,
            tc.tile_pool(name="apool", bufs=8) as apool,
            tc.tile_pool(name="hpool", bufs=2) as hpool,
            tc.tile_pool(name="h1pool", bufs=3) as h1pool,
            tc.tile_pool(name="h4pool", bufs=18) as h4pool,
            tc.tile_pool(name="spool", bufs=2) as spool,
            tc.tile_pool(name="colpool", bufs=8) as colpool,
            tc.tile_pool(name="epool", bufs=12) as epool,
            tc.tile_pool(name="psA", bufs=4, space="PSUM") as psA,
            tc.tile_pool(name="psB", bufs=2, space="PSUM") as psB,
            tc.tile_pool(name="psAcc", bufs=1, space="PSUM") as psAcc,
        ):
            # ---- chunk input prefetch (per-chunk x/A slices) ----
            xs: dict[int, object] = {}
            As: dict[int, object] = {}

            def prefetch(c):
                if not (0 <= c < nch) or c in xs:
                    return
                xt = xpool.tile([P // 2, 2, CH], F8, tag="x")
                nc.sync.dma_start(xt[:], xT_d.ap()[:, :, c * CH:(c + 1) * CH])
                at = apool.tile([P, FRT_PER_CH, S], F32, tag="A")
                nc.sync.dma_start(
                    at[:], A_d.ap()[:, c * FRT_PER_CH:(c + 1) * FRT_PER_CH, :])
                xs[c] = xt
                As[c] = at

            # ---- resident constants/weights.  DMA issue order tracks first
            # use: W1 + chunk 0/1 first, then each layer's weights in the
            # order the pipeline-fill steps consume them; tail-only
            # constants (mmc/rowm/b6/W6) last.
            W1s = wpool.tile([P // 2, 2, HID], F8, tag="W1")
            # startup: W1 on the (idle) scalar queue so it runs in
            # parallel with chunk 0/1's x/A DMAs on the SP queue
            nc.gpsimd.dma_start(W1s[:, :, :128], W1_d.ap()[:, :, :128])
            nc.gpsimd.dma_start(W1s[:, :, 128:], W1_d.ap()[:, :, 128:])
            prefetch(0)
            prefetch(1)
            W2q = [wpool.tile([P, 2, HID], F8, tag=f"W2q{j}", name=f"W2q{j}")
                   for j in range(KP)]
            W3q = [wpool.tile([P, 2, HID], F8, tag=f"W3q{j}", name=f"W3q{j}")
                   for j in range(KP)]
            W4q = [wpool.tile([P, 2, HID], F8, tag=f"W4q{j}", name=f"W4q{j}")
                   for j in range(KP)]
            for j in range(KP):
                nc.scalar.dma_start(W2q[j][:], W2_d.ap()[j])
            misc = wpool.tile([P, 256], F32, tag="misc")
            nc.sync.dma_start(misc[:], misc_d.ap())
            prefetch(2)
            for j in range(KP):
                nc.sync.dma_start(W3q[j][:], W3_d.ap()[j])
            b4qs = wpool.tile([1, 2, HID], F8, tag="b4q")
            nc.sync.dma_start(b4qs[:], b4_d.ap())
            onesq = wpool.tile([1, 2, P], F8, tag="onesq")
            nc.sync.dma_start(onesq[:], on_d.ap())
            for j in range(KP):
                nc.sync.dma_start(W4q[j][:], W4_d.ap()[j])
            W5s = wpool.tile([P, HID], BF16, tag="W5")
            nc.sync.dma_start(W5s[:], W5_d.ap())
            mmc = wpool.tile([P, P], F32R, tag="mmc")
            nc.sync.dma_start(mmc[:], mmc_d.ap())
            rowm = wpool.tile([1, 192], F32R, tag="rowm")
            nc.sync.dma_start(rowm[:], row_d.ap())
            b6s = wpool.tile([1, HID], F32R, tag="b6")
            nc.sync.dma_start(b6s[:], b6_d.ap())
            W6s = []
            for k in range(KS):
                t = wpool.tile([P, HID], F32R, tag=f"W6k{k}")
                nc.sync.dma_start(t[:], W6_d.ap()[k * P:(k + 1) * P, :])
                W6s.append(t)

            ones_col = mmc[:, MM_ONES:MM_ONES + 8]
            b5col = misc[:, MC_B5:MC_B5 + 1]
            idents = [misc[32 * q:32 * q + 32, MC_ID + 32 * q:MC_ID + 32 * q + 32]
                      for q in range(4)]
            W7v = mmc[:, MM_W7:MM_W7 + KS * NCLS].rearrange(
                "p (o c) -> p o c", c=NCLS)
            b7row = rowm[:, RW_B7:RW_B7 + NCLS]
            ones_row = rowm[:, RW_ONES:RW_ONES + P]

            # persistent PSUM pooled accumulators, one bank per hidden
            # half at partitions 0..31 (DoubleRow matmuls require dst
            # partition 0)
            pooled0 = psAcc.tile([S, 512], F32, tag="pooled0")
            pooled1 = psAcc.tile([S, 512], F32, tag="pooled1")
            # softmax denominator accumulator (f32r: feeds the tail matmul)
            eacc = wpool.tile([P, S], F32R, tag="eacc")

            # ---- per-chunk layer emitters (skewed pipeline below) ----
            h1s: dict[int, object] = {}
            h2s: dict[int, object] = {}
            h3s: dict[int, object] = {}
            h4s: dict[int, list] = {}
            pend: list[tuple] = []

            def do_L1(u, n_scalar_relu=0, ms=None, h1_cur=None):
                # fp8 DoubleRow over two 64-feature halves (b1 folded via
                # ones feature); relu on DVE (scalar split during pipeline
                # fill when the scalar engine is otherwise idle).  h1 is
                # stored as four per-DR-pair tiles [P, 2, CH] so each of
                # L2's j-matmuls depends only on the two relu slices it
                # actually reads.
                if ms is None:
                    ms = range(KS)
                xg = xs[u]
                if h1_cur is None:
                    h1_cur = [h1pool.tile([P, 2, CH], F8, tag=f"h1p{j}",
                                          name=f"h1p{j}") for j in range(KP)]
                h1 = h1_cur
                for m in ms:
                    ps = psA.tile([P, CH], F32, tag="mm")
                    for c2 in range(CH // NCOL):
                        xt = xg[:, :, c2 * NCOL:(c2 + 1) * NCOL]
                        nc.tensor.matmul(ps[:, c2 * NCOL:(c2 + 1) * NCOL],
                                         W1s[:, :, m * P:(m + 1) * P],
                                         xt, start=True, stop=True,
                                         perf_mode=DR)
                    if m < n_scalar_relu:
                        nc.scalar.activation(h1[m // 2][:, m % 2, :], ps[:],
                                             RELU)
                    else:
                        nc.vector.tensor_scalar_max(h1[m // 2][:, m % 2, :],
                                                    ps[:], 0.0)
                h1s[u] = h1
                if max(ms) == KS - 1:
                    xs.pop(u)
                return h1

            def do_L23(u, Wq, boff, li, ms=None, h_cur=None, alt=False):
                # fp8 DoubleRow; relu+bias on scalar (m<6) / DVE (m>=6)
                if ms is None:
                    ms = range(KS)
                h_in = h1s[u] if li == 2 else h2s[u]
                tag = "h2p" if li == 2 else "h3p"
                if h_cur is None:
                    h_cur = [hpool.tile([P, 2, CH], F8, tag=f"{tag}{j}",
                                        name=f"{tag}{j}") for j in range(KP)]
                h_out = h_cur
                for m in ms:
                    ps = psA.tile([P, CH], F32, tag="mm")
                    for c2 in range(CH // NCOL):
                        seg = ps[:, c2 * NCOL:(c2 + 1) * NCOL]
                        for j in range(KP):
                            nc.tensor.matmul(
                                seg, Wq[j][:, :, m * P:(m + 1) * P],
                                h_in[j][:, :, c2 * NCOL:(c2 + 1) * NCOL],
                                start=(j == 0), stop=(j == KP - 1),
                                perf_mode=DR)
                    bcol = misc[:, boff + m:boff + m + 1]
                    if (m % 2 == 1) if alt else (m >= 6):
                        nc.vector.tensor_scalar(
                            out=h_out[m // 2][:, m % 2, :], in0=ps[:],
                            scalar1=bcol, scalar2=0.0,
                            op0=ADD, op1=MAX)
                    else:
                        nc.scalar.activation(
                            h_out[m // 2][:, m % 2, :], ps[:], RELU,
                            bias=bcol)
                (h2s if li == 2 else h3s)[u] = h_out
                if max(ms) == KS - 1:
                    (h1s if li == 2 else h2s).pop(u)
                return h_out

            def do_L4(u, alt=False, fs=None):
                # frame-major fp8 DR; relu on scalar; h4 bf16 (score DVE 2x
                # rate + 1 cycle/row pooling matmuls)
                if fs is None:
                    fs = range(FRT_PER_CH)
                h3 = h3s[u]
                tiles = h4s.get(u, [])
                for f in fs:
                    h4 = h4pool.tile([P, HID], BF16, tag="h4")
                    for n in range(2):
                        ps4 = psB.tile([P, 512], F32, tag="l4")
                        for c2 in range(2):
                            seg = ps4[:, c2 * 256:(c2 + 1) * 256]
                            col0 = n * 512 + c2 * 256
                            for j in range(KP):
                                nc.tensor.matmul(
                                    seg,
                                    h3[j][:, :, f * P:(f + 1) * P],
                                    W4q[j][:, :, col0:col0 + 256],
                                    start=(j == 0), stop=False,
                                    perf_mode=DR)
                            nc.tensor.matmul(
                                seg, onesq[:],
                                b4qs[:, :, col0:col0 + 256],
                                start=False, stop=True, perf_mode=DR)
                        if alt and n == 1:
                            nc.vector.tensor_scalar_max(
                                h4[:, n * 512:(n + 1) * 512], ps4[:], 0.0)
                        else:
                            nc.scalar.activation(
                                h4[:, n * 512:(n + 1) * 512], ps4[:], RELU)
                    tiles.append(h4)
                h4s[u] = tiles
                if max(fs) == FRT_PER_CH - 1:
                    h3s.pop(u)

            def do_scores(u, step):
                # d = sum(h4*W5rep) via DVE mult (2x) + reduce (4x);
                # e = max(exp(d + b5), 1) on scalar/gpsimd; E = A*e and
                # eacc += E on gpsimd.  For the fp8 DoubleRow pooling, h4
                # is also requantized to fp8 PAIR tiles [128, 2, 1024]
                # (slot i = frame-tile 2p+i) on gpsimd — emitted FIRST so
                # the (long-ready) copies fill the Pool queue's idle front
                # while the et chain waits for the exp results.
                ag = As.pop(u)
                first_u = (u == 0)
                tiles = h4s.pop(u)
                h4q = []
                CPY = mybir.ActivationFunctionType.Copy
                for pi in range(FRT_PER_CH // 2):
                    hq = h4pool.tile([P, 2, HID], F8, tag="h4q")
                    if u >= nch - 1:
                        # drain: scalar engine is idle; freeing the Pool
                        # queue lets the final et chain -> pooling finish
                        # sooner
                        nc.scalar.activation(hq[:, 0, :], tiles[2 * pi][:],
                                             CPY)
                        nc.scalar.activation(hq[:, 1, :],
                                             tiles[2 * pi + 1][:], CPY)
                    else:
                        nc.gpsimd.tensor_scalar_add(hq[:, 0, :],
                                                    tiles[2 * pi][:], 0.0)
                        nc.gpsimd.tensor_scalar_add(hq[:, 1, :],
                                                    tiles[2 * pi + 1][:], 0.0)
                    h4q.append(hq)
                etp = None
                for f, h4 in enumerate(tiles):
                    prod = spool.tile([P, HID], BF16, tag="sc")
                    ct = colpool.tile([P, 16], F32, tag="col")
                    nc.vector.tensor_tensor(
                        out=prod[:], in0=h4[:], in1=W5s[:], op=MULT)
                    nc.vector.tensor_scalar(
                        out=prod[:], in0=prod[:], scalar1=1.0,
                        scalar2=0.0, op0=MULT, op1=ADD,
                        accum_out=ct[:, 0:1])
                    nc.scalar.activation(ct[:, 1:2], ct[:, 0:1], EXP,
                                         bias=b5col)
                    nc.gpsimd.tensor_scalar_max(ct[:, 2:3], ct[:, 1:2], 1.0)
                    if f % 2 == 0:
                        etp = epool.tile([P, 2, S], F8, tag="E")
                    nc.gpsimd.tensor_scalar_mul(
                        etp[:, f % 2, :], ag[:, f, :], ct[:, 2:3])
                    # eacc accumulates the SAME fp8-quantized E values the
                    # pooling numerator uses, so softmax weights sum to 1
                    if first_u and f == 0:
                        nc.gpsimd.tensor_scalar_add(eacc[:],
                                                    etp[:, 0, :], 0.0)
                    else:
                        nc.gpsimd.tensor_tensor(
                            out=eacc[:], in0=eacc.bitcast(F32)[:],
                            in1=etp[:, f % 2, :], op=ADD)
                    if f % 2 == 1:
                        st = bool(first_u and f == 1)
                        sp = bool(u == nch - 1 and f == FRT_PER_CH - 1)
                        pend.append((etp, h4q[f // 2], st, sp, step))

            def flush_pool(max_step, limit=1 << 30):
                # fp8 DoubleRow pooling matmuls (K=256: two frame-tiles per
                # pass) for score chains born at step <= max_step: 4
                # col-group quarter matmuls into the single pooled bank
                while pend and pend[0][4] <= max_step and limit > 0:
                    limit -= 1
                    etp, hq, st, sp, _ = pend.pop(0)
                    for h, pl in enumerate((pooled0, pooled1)):
                        # one full-row (N=512) fp8 DoubleRow matmul per
                        # hidden half (contraction = two frame-tiles)
                        nc.tensor.matmul(
                            pl[:], etp[:],
                            hq[:, :, 512 * h:512 * (h + 1)],
                            start=st, stop=sp, perf_mode=DR)

            # ---- main static pipeline over chunks ----
            # 4-deep skewed pipeline; chunk 0's L1 is pre-emitted with its
            # relus split across the still-idle scalar engine so step 1's
            # L2(0) never races the DVE queue.
            if nch >= 1:
                do_L1(0, n_scalar_relu=4)
            prefetch(3)
            for s in range(1, nch + 5):
                # interleave L1's m-pairs between the L2/L3 m-blocks so
                # psA banks filled by L1 are never drained back-to-back by
                # its own (DVE-serial) relus
                l1 = s < nch
                h1c = h2c = h3c = None
                if l1:
                    prefetch(s + PRE)
                if l1:
                    h1c = do_L1(s, ms=range(0, 2))
                if 0 <= s - 1 < nch:
                    h2c = do_L23(s - 1, W2q, MC_B2, 2, ms=range(0, 4),
                                 alt=(s - 1 >= nch - 2))
                if l1:
                    do_L1(s, ms=range(2, 4), h1_cur=h1c)
                if 0 <= s - 1 < nch:
                    do_L23(s - 1, W2q, MC_B2, 2, ms=range(4, 8), h_cur=h2c,
                           alt=(s - 1 >= nch - 2))
                if l1:
                    do_L1(s, ms=range(4, 6), h1_cur=h1c)
                if 0 <= s - 2 < nch:
                    h3c = do_L23(s - 2, W3q, MC_B3, 3, ms=range(0, 4),
                                 alt=(s - 2 >= nch - 2))
                if l1:
                    do_L1(s, ms=range(6, 8), h1_cur=h1c)
                if 0 <= s - 2 < nch:
                    do_L23(s - 2, W3q, MC_B3, 3, ms=range(4, 8), h_cur=h3c,
                           alt=(s - 2 >= nch - 2))
                if 0 <= s - 3 < nch:
                    do_L4(s - 3, alt=(s - 3 >= nch - 2), fs=(0, 1))
                    flush_pool(s - 2)
                    do_L4(s - 3, alt=(s - 3 >= nch - 2), fs=(2, 3))
                else:
                    flush_pool(s - 2)
                if 0 <= s - 4 < nch:
                    do_scores(s - 4, s)
                if s >= nch:
                    # pipeline drain: no L1..L4 work left to hide behind,
                    # flush pooling as soon as the score chain lands
                    flush_pool(s)
            flush_pool(1 << 30)

            # ---- tail: final per-utterance MLP (f32r) ----
            # denom[s] = sum_p eacc[p, s] via one tiny PE matmul -> [S, 1]
            psd = psA.tile([S, 8], F32, tag="mm")
            nc.tensor.matmul(psd[:], eacc[:], ones_col,
                             start=True, stop=True)
            fc = colpool.tile([P, 16], F32, tag="col")
            nc.vector.tensor_copy(out=fc[:S, 0:1], in_=psd[:, 0:1])
            nc.vector.reciprocal(fc[:S, 1:2], fc[:S, 0:1])
            # denom as an f32r row [1, S] (rank-1 b6*denom matmul lhsT)
            psr = psA.tile([1, S], F32, tag="mm")
            nc.tensor.transpose(psr[:], fc[:S, 0:1], idents[0])
            drow = wpool.tile([1, S], F32R, tag="drow")
            nc.vector.tensor_copy(out=drow[:], in_=psr[:])

            # pooled PSUM -> SBUF (unscaled; normalization is folded
            # into the W6 activation below)
            pooled_sb = wpool.tile([S, HID], F32, tag="poolsb")
            nc.vector.tensor_copy(out=pooled_sb[:, :512], in_=pooled0[:])
            nc.vector.tensor_copy(out=pooled_sb[:, 512:], in_=pooled1[:])

            # transpose pooled -> pooledT [hid, seg], software-pipelined with
            # the W6 matmuls: psg(n) accumulates k-subtile products as soon
            # as pooledT[:, k] lands
            tposed = wpool.tile([P, KS, 2 * S], F32R, tag="tposed")
            pooledT = tposed[:, :, :S]
            gT = tposed[:, :, S:]
            psg0 = psB.tile([S, 512], F32, tag="l4")
            psg1 = psB.tile([S, 512], F32, tag="l4")
            for k in range(KS):
                pst = psA.tile([P, S], F32, tag="mm")
                nc.tensor.transpose(
                    pst[:], pooled_sb[:, 128 * k:128 * (k + 1)], idents[0])
                nc.vector.tensor_copy(out=pooledT[:, k, :], in_=pst[:])
                nc.tensor.matmul(psg0[:], pooledT[:, k, :],
                                 W6s[k][:, :512],
                                 start=(k == 0), stop=False)
                nc.tensor.matmul(psg1[:], pooledT[:, k, :],
                                 W6s[k][:, 512:],
                                 start=(k == 0), stop=False)

            # g = relu((pooled @ W6 + denom*b6) / denom)  (seg-major [S, HID])
            # drow = denom row: psg accumulates pooled@W6 + denom*b6
            g_sb = spool.tile([S, HID], F32, tag="sc")
            for n, psg in enumerate((psg0, psg1)):
                nc.tensor.matmul(psg[:], drow[:],
                                 b6s[:, n * 512:(n + 1) * 512],
                                 start=False, stop=True)
                nc.scalar.activation(g_sb[:, n * 512:(n + 1) * 512], psg[:],
                                     RELU, scale=fc[:S, 1:2])

            # gT [hid, seg]
            for k in range(KS):
                pst = psA.tile([P, S], F32, tag="mm")
                nc.tensor.transpose(pst[:], g_sb[:, k * P:(k + 1) * P],
                                    idents[0])
                nc.vector.tensor_copy(out=gT[:, k, :], in_=pst[:])

            # out = g @ W7 + b7
            pso = psA.tile([S, NCLS], F32, tag="mm")
            for k in range(KS):
                nc.tensor.matmul(pso[:], gT[:, k, :], W7v[:, k, :],
                                 start=(k == 0), stop=False)
            nc.tensor.matmul(pso[:], ones_row[:, :S], b7row,
                             start=False, stop=True)
            oc = colpool.tile([S, 16], F32, tag="col")
            nc.vector.tensor_copy(out=oc[:, :NCLS], in_=pso[:])
            nc.sync.dma_start(out_d.ap()[:], oc[:, :NCLS])

    nc.compile()
    return nc


def _q8(a: np.ndarray) -> np.ndarray:
    return np.asarray(a, dtype=np.float32).astype(E4NP)


def _pack_dr(W: np.ndarray) -> np.ndarray:
    """[1024, N] weight matrix -> DoubleRow fp8 layout [KP, 128, 2, N]."""
    return np.ascontiguousarray(
        _q8(W).reshape(KP, 2, P, -1).transpose(0, 2, 1, 3))


def prepare_inputs(x, W1, b1, W2, b2, W3, b3, W4, b4, W5, b5, W6, b6, W7, b7,
                   lengths):
    """Host-side sharding/packing. Returns (in_maps, bins, m_pad)."""
    x = np.ascontiguousarray(np.asarray(x, dtype=np.float32))
    lengths = np.asarray(lengths)
    total = x.shape[0]
    seg_ids = _segment_ids(lengths, total)
    counts = np.bincount(seg_ids, minlength=NSEG).astype(np.int64)
    starts = np.zeros(NSEG + 1, dtype=np.int64)
    starts[1:] = np.cumsum(counts)

    bins = _balance_segments(counts)
    core_frames = [int(sum(counts[s] for s in b)) for b in bins]
    m_pad = ((max(core_frames) + CH - 1) // CH) * CH
    frt = m_pad // P

    W1p = np.zeros((P, HID), dtype=np.float32)
    W1p[:FEAT] = np.asarray(W1, dtype=np.float32)
    W1p[FEAT] = np.asarray(b1, dtype=np.float32)
    # DoubleRow over two 64-feature halves: [64, 2, HID]
    W1q = np.ascontiguousarray(
        _q8(W1p).reshape(2, P // 2, HID).transpose(1, 0, 2))

    misc = np.zeros((P, 256), dtype=np.float32)
    misc[:, MC_B2:MC_B2 + KS] = np.asarray(b2, np.float32).reshape(KS, P).T
    misc[:, MC_B3:MC_B3 + KS] = np.asarray(b3, np.float32).reshape(KS, P).T
    misc[:, MC_B5] = np.float32(np.asarray(b5, np.float32).reshape(-1)[0])
    for q in range(4):
        misc[32 * q:32 * q + 32, MC_ID + 32 * q:MC_ID + 32 * q + 32] = np.eye(
            32, dtype=np.float32)

    mmcc = np.zeros((P, P), dtype=np.float32)
    mmcc[:, MM_ONES:MM_ONES + 8] = 1.0
    mmcc[:, MM_W7:MM_W7 + KS * NCLS] = np.asarray(W7, np.float32).reshape(
        KS, P, NCLS).transpose(1, 0, 2).reshape(P, KS * NCLS)

    rowm = np.zeros((1, 192), dtype=np.float32)
    rowm[0, RW_ONES:RW_ONES + P] = 1.0
    rowm[0, RW_B7:RW_B7 + NCLS] = np.asarray(b7, np.float32).reshape(-1)

    b4q = np.zeros((1, 2, HID), dtype=E4NP)
    b4q[0, 0, :] = _q8(np.asarray(b4, np.float32).reshape(-1))
    onesq = np.zeros((1, 2, P), dtype=E4NP)
    onesq[0, 0, :] = np.float32(1.0)

    shared = dict(
        W1q=W1q,
        W2q=_pack_dr(np.asarray(W2, np.float32)),
        W3q=_pack_dr(np.asarray(W3, np.float32)),
        W4q=_pack_dr(np.asarray(W4, np.float32)),
        W5rep=np.broadcast_to(
            np.asarray(W5, np.float32).reshape(1, HID).astype(
                ml_dtypes.bfloat16), (P, HID)).copy(),
        W6=np.ascontiguousarray(np.asarray(W6, np.float32)),
        b4q=b4q,
        onesq=onesq,
        b6r=np.asarray(b6, np.float32).reshape(1, HID),
        miscc=misc,
        mmcc=mmcc,
        rowm=rowm,
    )

    in_maps = []
    for core in range(NCORES):
        segs = bins[core]
        xs = [x[starts[s]:starts[s + 1]] for s in segs]
        xcat = np.concatenate(xs, axis=0) if xs else np.zeros((0, FEAT), np.float32)
        n = xcat.shape[0]
        xT = np.zeros((P, m_pad), dtype=np.float32)
        xT[:FEAT, :n] = xcat.T
        xT[FEAT, :n] = 1.0  # constant feature -> b1
        A = np.zeros((m_pad, SEGS_PER_CORE), dtype=np.float32)
        off = 0
        for j, s in enumerate(segs):
            ln = int(counts[s])
            A[off:off + ln, j] = 1.0
            off += ln
        im = dict(shared)
        # fp8 x, DoubleRow halves: xq[p, i, col] = xpad[i*64 + p, col]
        im["xq"] = np.ascontiguousarray(
            _q8(xT).reshape(2, P // 2, m_pad).transpose(1, 0, 2))
        # partition-major layout [P, frt, S]: Ah[p, t, s] = A[t*128 + p, s]
        im["Amat"] = np.ascontiguousarray(
            A.reshape(frt, P, SEGS_PER_CORE).transpose(1, 0, 2))
        in_maps.append(im)
    return in_maps, bins, m_pad


_PROGRAM_CACHE: dict[int, object] = {}


def kernel(**inputs) -> np.ndarray:
    in_maps, bins, m_pad = prepare_inputs(**inputs)
    nc = _PROGRAM_CACHE.get(m_pad)
    if nc is None:
        nc = _build_program(m_pad)
        _PROGRAM_CACHE[m_pad] = nc
    res = run_bass_kernel_spmd(nc, in_maps, core_ids=list(range(NCORES)))
    out = np.zeros((NSEG, NCLS), dtype=np.float32)
    for core in range(NCORES):
        out[bins[core]] = res.results[core]["out"]
    return out


# revision 83
# speedup vs baseline: 1.0160x; 1.0156x over previous
"""Trainium2 Bass kernel for nn_Dnn_with_Attention (ragged attention-pooled DNN).

Contract: kernel(**inputs) takes FULL unsharded numpy inputs (keys as in
reference.setup_inputs()) and returns the FULL [256, 10] float32 output.

Strategy (data-parallel over utterances, 8 NeuronCores):
  - Host: balance the 256 segments over 8 cores (32 whole segments each,
    greedy LPT + swap refinement -> every core at exactly sum/8 frames on
    the benchmark lengths, so m_pad = 16384 = 32 chunks), gather each
    core's frames, pack x as fp8 feature-major pairs [64, 2, M_PAD] and
    build a per-frame one-hot segment membership matrix A.  A row of ones
    is appended as feature 78 so b1 folds into W1.  All four weight
    matrices are pre-quantized to fp8 e4m3 in DoubleRow pair layout.
  - Device (per core): ALL four layers run as fp8(e4m3) DoubleRow matmuls
    (two 128-row contraction tiles per pass); inter-layer activations are
    stored e4m3, h4 in bf16.  b4 is added via a tiny fp8 ones-row
    DoubleRow matmul inside the same PSUM accumulation group.
  - A fully static (no hardware loop) skewed software pipeline over
    512-frame chunks: at step s the PE runs L2(s-1) and L3(s-2) in 4-wide
    m-blocks with L1(s)'s m-pairs INTERLEAVED between them (so psA banks
    filled by L1 are never drained back-to-back by its own DVE-serial
    relus), then L4(s-3) in two halves with the (deferred, s-6)
    pooling matmuls between them, and scores(s-4).  x/A are
    DMA-prefetched per chunk 3 steps ahead; W1/W2 stream on the scalar
    engine's DMA queue at startup, in parallel with chunk 0's inputs.
  - Activations: L2/L3 m<6 + all L4 relus on scalar (bias via the
    activation unit), L1 relus + L2/L3 m>=6 on DVE; the score reduction
    is a DVE 2x tensor_tensor multiply + 4x tensor_scalar accumulate;
    exp on scalar; clamp/mask/eacc on gpsimd.  During the pipeline drain
    the last chunks alternate act engines per m-tile (latency, not
    throughput, binds there) and the last chunk's h4 requant runs as
    scalar Copy activations.
  - Segment softmax pooling as fp8 DoubleRow PE matmuls contracting TWO
    128-frame tiles per pass (a DR pair (k, k+128) is the SAME partition
    of two consecutive frame-tiles, so E and a gpsimd-requantized fp8
    copy of h4 are stored as pair tiles [128, 2, *] with no partition
    shuffle), accumulated into two persistent PSUM banks (DR matmuls
    require dst partition 0), deferred two steps behind the score chain.
    The softmax denominator is an SBUF f32r accumulator (eacc += E on
    gpsimd, the same quantized values the numerator uses) reduced by one
    tiny matmul in the tail.
  - Tail: the final per-utterance MLP runs once in f32r; the softmax
    normalization is folded into the W6 activation (per-partition
    scale = 1/denom; b6 enters pre-scaled by denom via a rank-1 matmul
    with the denominator row), and the pooled->pooledT transposes are
    software-pipelined with the W6 k-subtile matmuls.
"""

import sys

sys.path.insert(0, "/opt/trn_rl_repo")

import numpy as np
import ml_dtypes

import concourse.bass as bass
import concourse.mybir as mybir
import concourse.tile as tile
from concourse import bacc
from concourse.bass_utils import run_bass_kernel_spmd

P = 128
FEAT = 78
HID = 1024
NCLS = 10
NSEG = 256
NCORES = 8
SEGS_PER_CORE = NSEG // NCORES
CH = 512           # frames per chunk (free dim of layer-1..3 matmuls)
FRT_PER_CH = CH // P
NCOL = 256         # moving-dim columns per DoubleRow matmul (HW limit)
KS = HID // P      # 8 k-subtiles
KP = KS // 2       # 4 DoubleRow k-pairs
F32 = mybir.dt.float32
F32R = mybir.dt.float32r
F8 = mybir.dt.float8e4
BF16 = mybir.dt.bfloat16
E4NP = ml_dtypes.float8_e4m3

# misc constant tile column layout ([128, 256] f32, host-packed)
MC_B2 = 0          # cols 0..7   : b2 striped [128, 8]
MC_B3 = 8          # cols 8..15  : b3 striped
MC_B5 = 17         # col 17      : b5 replicated down partitions
MC_B7BC = 96       # cols 96..105, rows 0..31: b7 broadcast to 32 rows
MC_ID = 128        # cols 128..255: four 32x32 identity blocks, block q at
                   # rows 32q..32q+32, cols 128+32q..128+32q+32
# f32r matmul-constants tile ([128, 128])
MM_ONES = 0        # cols 0..7   : ones columns (denom matmul rhs, N=8)
MM_W7 = 16         # cols 16..95 : W7 as [128, 8, 10]
# row constants tile ([1, 192] f32r, host-packed)
RW_ONES = 0        # cols 0..127 : ones row
RW_B7 = 128        # cols 128..137 : b7


def _segment_ids(lengths: np.ndarray, total: int) -> np.ndarray:
    """Replicate jnp.repeat(arange(n), lengths, total_repeat_length=total)."""
    lengths = np.asarray(lengths, dtype=np.int64)
    seg = np.repeat(np.arange(lengths.shape[0], dtype=np.int32), np.maximum(lengths, 0))
    if seg.shape[0] >= total:
        return seg[:total]
    pad_val = seg[-1] if seg.shape[0] > 0 else np.int32(0)
    return np.concatenate([seg, np.full(total - seg.shape[0], pad_val, np.int32)])


def _balance_segments(lengths: np.ndarray) -> list[list[int]]:
    """Assign 256 segments to 8 cores, 32 each, minimizing max frame count.

    Greedy LPT init + single-move / pairwise-swap local search.  On the
    benchmark lengths this reaches a perfect partition (all cores at
    sum/8 frames), saving one 512-frame chunk of padding.
    """
    lengths = np.asarray(lengths, dtype=np.int64)
    cap = SEGS_PER_CORE
    order = np.argsort(-lengths, kind="stable")
    loads = [0] * NCORES
    bins: list[list[int]] = [[] for _ in range(NCORES)]
    for s in order:
        cands = [c for c in range(NCORES) if len(bins[c]) < cap]
        c = min(cands, key=lambda c: (loads[c], c))
        bins[c].append(int(s))
        loads[c] += int(lengths[s])
    target = (int(lengths.sum()) + NCORES - 1) // NCORES
    for _ in range(4096):
        hi = max(range(NCORES), key=lambda c: loads[c])
        if loads[hi] <= target:
            break
        best = None
        for other in range(NCORES):
            if other == hi:
                continue
            if len(bins[other]) < cap:
                for i, s in enumerate(bins[hi]):
                    d = int(lengths[s])
                    newmax = max(loads[hi] - d, loads[other] + d)
                    if best is None or newmax < best[0]:
                        best = (newmax, hi, other, i, None)
            for i, s1 in enumerate(bins[hi]):
                d1 = int(lengths[s1])
                for j, s2 in enumerate(bins[other]):
                    d2 = int(lengths[s2])
                    if d1 <= d2:
                        continue
                    newmax = max(loads[hi] - d1 + d2, loads[other] + d1 - d2)
                    if best is None or newmax < best[0]:
                        best = (newmax, hi, other, i, j)
        if best is None or best[0] >= loads[hi]:
            break
        _, a, b, i, j = best
        s1 = bins[a].pop(i)
        if j is None:
            bins[b].append(s1)
            loads[a] -= int(lengths[s1])
            loads[b] += int(lengths[s1])
        else:
            s2 = bins[b][j]
            bins[b][j] = s1
            bins[a].append(s2)
            loads[a] += int(lengths[s2]) - int(lengths[s1])
            loads[b] += int(lengths[s1]) - int(lengths[s2])
    for b in bins:
        b.sort()
    return bins


PRE = 3            # chunk DMA prefetch distance (steps)


def _build_program(m_pad: int):
    """Emit the fully static Bass/Tile program for one core (m_pad frames)."""
    nch = m_pad // CH
    frt = m_pad // P
    S = SEGS_PER_CORE

    nc = bacc.Bacc("TRN2", target_bir_lowering=False, debug=False,
                   num_devices=NCORES)

    xT_d = nc.dram_tensor("xq", [P // 2, 2, m_pad], F8, kind="ExternalInput")
    A_d = nc.dram_tensor("Amat", [P, frt, S], F32, kind="ExternalInput")
    W1_d = nc.dram_tensor("W1q", [P // 2, 2, HID], F8, kind="ExternalInput")
    W2_d = nc.dram_tensor("W2q", [KP, P, 2, HID], F8, kind="ExternalInput")
    W3_d = nc.dram_tensor("W3q", [KP, P, 2, HID], F8, kind="ExternalInput")
    W4_d = nc.dram_tensor("W4q", [KP, P, 2, HID], F8, kind="ExternalInput")
    W5_d = nc.dram_tensor("W5rep", [P, HID], BF16, kind="ExternalInput")
    W6_d = nc.dram_tensor("W6", [HID, HID], BF16, kind="ExternalInput")
    W7b_d = nc.dram_tensor("W7b", [P, KS * NCLS], BF16, kind="ExternalInput")
    b4_d = nc.dram_tensor("b4q", [1, 2, HID], F8, kind="ExternalInput")
    on_d = nc.dram_tensor("onesq", [1, 2, P], F8, kind="ExternalInput")
    b6_d = nc.dram_tensor("b6r", [1, HID], F32R, kind="ExternalInput")
    misc_d = nc.dram_tensor("miscc", [P, 256], F32, kind="ExternalInput")
    mmc_d = nc.dram_tensor("mmcc", [P, P], F32R, kind="ExternalInput")
    row_d = nc.dram_tensor("rowm", [1, 192], F32R, kind="ExternalInput")
    out_d = nc.dram_tensor("out", [S, NCLS], F32, kind="ExternalOutput")

    RELU = mybir.ActivationFunctionType.Relu
    EXP = mybir.ActivationFunctionType.Exp
    MULT = mybir.AluOpType.mult
    ADD = mybir.AluOpType.add
    MAX = mybir.AluOpType.max
    DR = mybir.MatmulPerfMode.DoubleRow

    with tile.TileContext(nc) as tc:
        with (
            tc.tile_pool(name="wpool", bufs=1) as wpool,
            tc.tile_pool(name="xpool", bufs=4) as xpool,
            tc.tile_pool(name="apool", bufs=8) as apool,
            tc.tile_pool(name="hpool", bufs=2) as hpool,
            tc.tile_pool(name="h1pool", bufs=3) as h1pool,
            tc.tile_pool(name="h4pool", bufs=18) as h4pool,
            tc.tile_pool(name="spool", bufs=2) as spool,
            tc.tile_pool(name="colpool", bufs=8) as colpool,
            tc.tile_pool(name="epool", bufs=12) as epool,
            tc.tile_pool(name="psA", bufs=4, space="PSUM") as psA,
            tc.tile_pool(name="psB", bufs=2, space="PSUM") as psB,
            tc.tile_pool(name="psAcc", bufs=1, space="PSUM") as psAcc,
        ):
            # ---- chunk input prefetch (per-chunk x/A slices) ----
            xs: dict[int, object] = {}
            As: dict[int, object] = {}

            def prefetch(c):
                if not (0 <= c < nch) or c in xs:
                    return
                xt = xpool.tile([P // 2, 2, CH], F8, tag="x")
                nc.sync.dma_start(xt[:], xT_d.ap()[:, :, c * CH:(c + 1) * CH])
                at = apool.tile([P, FRT_PER_CH, S], F32, tag="A")
                nc.sync.dma_start(
                    at[:], A_d.ap()[:, c * FRT_PER_CH:(c + 1) * FRT_PER_CH, :])
                xs[c] = xt
                As[c] = at

            # ---- resident constants/weights.  DMA issue order tracks first
            # use: W1 + chunk 0/1 first, then each layer's weights in the
            # order the pipeline-fill steps consume them; tail-only
            # constants (mmc/rowm/b6/W6) last.
            W1s = wpool.tile([P // 2, 2, HID], F8, tag="W1")
            # startup: W1 on the (idle) scalar queue so it runs in
            # parallel with chunk 0/1's x/A DMAs on the SP queue
            nc.gpsimd.dma_start(W1s[:, :, :128], W1_d.ap()[:, :, :128])
            nc.gpsimd.dma_start(W1s[:, :, 128:], W1_d.ap()[:, :, 128:])
            prefetch(0)
            prefetch(1)
            W2q = [wpool.tile([P, 2, HID], F8, tag=f"W2q{j}", name=f"W2q{j}")
                   for j in range(KP)]
            W3q = [wpool.tile([P, 2, HID], F8, tag=f"W3q{j}", name=f"W3q{j}")
                   for j in range(KP)]
            W4q = [wpool.tile([P, 2, HID], F8, tag=f"W4q{j}", name=f"W4q{j}")
                   for j in range(KP)]
            for j in range(KP):
                nc.scalar.dma_start(W2q[j][:], W2_d.ap()[j])
            misc = wpool.tile([P, 256], F32, tag="misc")
            nc.sync.dma_start(misc[:], misc_d.ap())
            prefetch(2)
            for j in range(KP):
                nc.sync.dma_start(W3q[j][:], W3_d.ap()[j])
            b4qs = wpool.tile([1, 2, HID], F8, tag="b4q")
            nc.sync.dma_start(b4qs[:], b4_d.ap())
            onesq = wpool.tile([1, 2, P], F8, tag="onesq")
            nc.sync.dma_start(onesq[:], on_d.ap())
            for j in range(KP):
                nc.sync.dma_start(W4q[j][:], W4_d.ap()[j])
            W5s = wpool.tile([P, HID], BF16, tag="W5")
            nc.sync.dma_start(W5s[:], W5_d.ap())
            mmc = wpool.tile([P, P], F32R, tag="mmc")
            nc.sync.dma_start(mmc[:], mmc_d.ap())
            rowm = wpool.tile([1, 192], F32R, tag="rowm")
            nc.sync.dma_start(rowm[:], row_d.ap())
            b6s = wpool.tile([1, HID], F32R, tag="b6")
            nc.sync.dma_start(b6s[:], b6_d.ap())
            W6s = []
            for k in range(KS):
                t = wpool.tile([P, HID], BF16, tag=f"W6k{k}")
                nc.sync.dma_start(t[:], W6_d.ap()[k * P:(k + 1) * P, :])
                W6s.append(t)
            W7b = wpool.tile([P, KS * NCLS], BF16, tag="W7b")
            nc.sync.dma_start(W7b[:], W7b_d.ap())
            W7bv = W7b.rearrange("p (o c) -> p o c", c=NCLS)

            ones_col = mmc[:, MM_ONES:MM_ONES + 8]
            b5col = misc[:, MC_B5:MC_B5 + 1]
            idents = [misc[32 * q:32 * q + 32, MC_ID + 32 * q:MC_ID + 32 * q + 32]
                      for q in range(4)]
            W7v = mmc[:, MM_W7:MM_W7 + KS * NCLS].rearrange(
                "p (o c) -> p o c", c=NCLS)
            b7row = rowm[:, RW_B7:RW_B7 + NCLS]
            ones_row = rowm[:, RW_ONES:RW_ONES + P]

            # persistent PSUM pooled accumulators, one bank per hidden
            # half at partitions 0..31 (DoubleRow matmuls require dst
            # partition 0)
            pooled0 = psAcc.tile([S, 512], F32, tag="pooled0")
            pooled1 = psAcc.tile([S, 512], F32, tag="pooled1")
            # softmax denominator accumulator (f32r: feeds the tail matmul)
            eacc = wpool.tile([P, S], F32R, tag="eacc")

            # ---- per-chunk layer emitters (skewed pipeline below) ----
            h1s: dict[int, object] = {}
            h2s: dict[int, object] = {}
            h3s: dict[int, object] = {}
            h4s: dict[int, list] = {}
            pend: list[tuple] = []

            def do_L1(u, n_scalar_relu=0, ms=None, h1_cur=None):
                # fp8 DoubleRow over two 64-feature halves (b1 folded via
                # ones feature); relu on DVE (scalar split during pipeline
                # fill when the scalar engine is otherwise idle).  h1 is
                # stored as four per-DR-pair tiles [P, 2, CH] so each of
                # L2's j-matmuls depends only on the two relu slices it
                # actually reads.
                if ms is None:
                    ms = range(KS)
                xg = xs[u]
                if h1_cur is None:
                    h1_cur = [h1pool.tile([P, 2, CH], F8, tag=f"h1p{j}",
                                          name=f"h1p{j}") for j in range(KP)]
                h1 = h1_cur
                for m in ms:
                    ps = psA.tile([P, CH], F32, tag="mm")
                    for c2 in range(CH // NCOL):
                        xt = xg[:, :, c2 * NCOL:(c2 + 1) * NCOL]
                        nc.tensor.matmul(ps[:, c2 * NCOL:(c2 + 1) * NCOL],
                                         W1s[:, :, m * P:(m + 1) * P],
                                         xt, start=True, stop=True,
                                         perf_mode=DR)
                    if m < n_scalar_relu:
                        nc.scalar.activation(h1[m // 2][:, m % 2, :], ps[:],
                                             RELU)
                    else:
                        nc.vector.tensor_scalar_max(h1[m // 2][:, m % 2, :],
                                                    ps[:], 0.0)
                h1s[u] = h1
                if max(ms) == KS - 1:
                    xs.pop(u)
                return h1

            def do_L23(u, Wq, boff, li, ms=None, h_cur=None, alt=False):
                # fp8 DoubleRow; relu+bias on scalar (m<6) / DVE (m>=6)
                if ms is None:
                    ms = range(KS)
                h_in = h1s[u] if li == 2 else h2s[u]
                tag = "h2p" if li == 2 else "h3p"
                if h_cur is None:
                    h_cur = [hpool.tile([P, 2, CH], F8, tag=f"{tag}{j}",
                                        name=f"{tag}{j}") for j in range(KP)]
                h_out = h_cur
                for m in ms:
                    ps = psA.tile([P, CH], F32, tag="mm")
                    for c2 in range(CH // NCOL):
                        seg = ps[:, c2 * NCOL:(c2 + 1) * NCOL]
                        for j in range(KP):
                            nc.tensor.matmul(
                                seg, Wq[j][:, :, m * P:(m + 1) * P],
                                h_in[j][:, :, c2 * NCOL:(c2 + 1) * NCOL],
                                start=(j == 0), stop=(j == KP - 1),
                                perf_mode=DR)
                    bcol = misc[:, boff + m:boff + m + 1]
                    if (m % 2 == 1) if alt else (m >= 6):
                        nc.vector.tensor_scalar(
                            out=h_out[m // 2][:, m % 2, :], in0=ps[:],
                            scalar1=bcol, scalar2=0.0,
                            op0=ADD, op1=MAX)
                    else:
                        nc.scalar.activation(
                            h_out[m // 2][:, m % 2, :], ps[:], RELU,
                            bias=bcol)
                (h2s if li == 2 else h3s)[u] = h_out
                if max(ms) == KS - 1:
                    (h1s if li == 2 else h2s).pop(u)
                return h_out

            def do_L4(u, alt=False, fs=None):
                # frame-major fp8 DR; relu on scalar; h4 bf16 (score DVE 2x
                # rate + 1 cycle/row pooling matmuls)
                if fs is None:
                    fs = range(FRT_PER_CH)
                h3 = h3s[u]
                tiles = h4s.get(u, [])
                for f in fs:
                    h4 = h4pool.tile([P, HID], BF16, tag="h4")
                    for n in range(2):
                        ps4 = psB.tile([P, 512], F32, tag="l4")
                        for c2 in range(2):
                            seg = ps4[:, c2 * 256:(c2 + 1) * 256]
                            col0 = n * 512 + c2 * 256
                            for j in range(KP):
                                nc.tensor.matmul(
                                    seg,
                                    h3[j][:, :, f * P:(f + 1) * P],
                                    W4q[j][:, :, col0:col0 + 256],
                                    start=(j == 0), stop=False,
                                    perf_mode=DR)
                            nc.tensor.matmul(
                                seg, onesq[:],
                                b4qs[:, :, col0:col0 + 256],
                                start=False, stop=True, perf_mode=DR)
                        if alt and n == 1:
                            nc.vector.tensor_scalar_max(
                                h4[:, n * 512:(n + 1) * 512], ps4[:], 0.0)
                        else:
                            nc.scalar.activation(
                                h4[:, n * 512:(n + 1) * 512], ps4[:], RELU)
                    tiles.append(h4)
                h4s[u] = tiles
                if max(fs) == FRT_PER_CH - 1:
                    h3s.pop(u)

            def do_scores(u, step):
                # d = sum(h4*W5rep) via DVE mult (2x) + reduce (4x);
                # e = max(exp(d + b5), 1) on scalar/gpsimd; E = A*e and
                # eacc += E on gpsimd.  For the fp8 DoubleRow pooling, h4
                # is also requantized to fp8 PAIR tiles [128, 2, 1024]
                # (slot i = frame-tile 2p+i) on gpsimd — emitted FIRST so
                # the (long-ready) copies fill the Pool queue's idle front
                # while the et chain waits for the exp results.
                ag = As.pop(u)
                first_u = (u == 0)
                tiles = h4s.pop(u)
                h4q = []
                CPY = mybir.ActivationFunctionType.Copy
                for pi in range(FRT_PER_CH // 2):
                    hq = h4pool.tile([P, 2, HID], F8, tag="h4q")
                    if u >= nch - 1:
                        # drain: scalar engine is idle; freeing the Pool
                        # queue lets the final et chain -> pooling finish
                        # sooner
                        nc.scalar.activation(hq[:, 0, :], tiles[2 * pi][:],
                                             CPY)
                        nc.scalar.activation(hq[:, 1, :],
                                             tiles[2 * pi + 1][:], CPY)
                    else:
                        nc.gpsimd.tensor_scalar_add(hq[:, 0, :],
                                                    tiles[2 * pi][:], 0.0)
                        nc.gpsimd.tensor_scalar_add(hq[:, 1, :],
                                                    tiles[2 * pi + 1][:], 0.0)
                    h4q.append(hq)
                etp = None
                for f, h4 in enumerate(tiles):
                    prod = spool.tile([P, HID], BF16, tag="sc")
                    ct = colpool.tile([P, 16], F32, tag="col")
                    nc.vector.tensor_tensor(
                        out=prod[:], in0=h4[:], in1=W5s[:], op=MULT)
                    nc.vector.tensor_scalar(
                        out=prod[:], in0=prod[:], scalar1=1.0,
                        scalar2=0.0, op0=MULT, op1=ADD,
                        accum_out=ct[:, 0:1])
                    nc.scalar.activation(ct[:, 1:2], ct[:, 0:1], EXP,
                                         bias=b5col)
                    nc.gpsimd.tensor_scalar_max(ct[:, 2:3], ct[:, 1:2], 1.0)
                    if f % 2 == 0:
                        etp = epool.tile([P, 2, S], F8, tag="E")
                    nc.gpsimd.tensor_scalar_mul(
                        etp[:, f % 2, :], ag[:, f, :], ct[:, 2:3])
                    # eacc accumulates the SAME fp8-quantized E values the
                    # pooling numerator uses, so softmax weights sum to 1
                    if first_u and f == 0:
                        nc.gpsimd.tensor_scalar_add(eacc[:],
                                                    etp[:, 0, :], 0.0)
                    else:
                        nc.gpsimd.tensor_tensor(
                            out=eacc[:], in0=eacc.bitcast(F32)[:],
                            in1=etp[:, f % 2, :], op=ADD)
                    if f % 2 == 1:
                        st = bool(first_u and f == 1)
                        sp = bool(u == nch - 1 and f == FRT_PER_CH - 1)
                        pend.append((etp, h4q[f // 2], st, sp, step))

            def flush_pool(max_step, limit=1 << 30):
                # fp8 DoubleRow pooling matmuls (K=256: two frame-tiles per
                # pass) for score chains born at step <= max_step: 4
                # col-group quarter matmuls into the single pooled bank
                while pend and pend[0][4] <= max_step and limit > 0:
                    limit -= 1
                    etp, hq, st, sp, _ = pend.pop(0)
                    for h, pl in enumerate((pooled0, pooled1)):
                        # one full-row (N=512) fp8 DoubleRow matmul per
                        # hidden half (contraction = two frame-tiles)
                        nc.tensor.matmul(
                            pl[:], etp[:],
                            hq[:, :, 512 * h:512 * (h + 1)],
                            start=st, stop=sp, perf_mode=DR)

            # ---- main static pipeline over chunks ----
            # 4-deep skewed pipeline; chunk 0's L1 is pre-emitted with its
            # relus split across the still-idle scalar engine so step 1's
            # L2(0) never races the DVE queue.
            if nch >= 1:
                do_L1(0, n_scalar_relu=4)
            prefetch(3)
            for s in range(1, nch + 5):
                # interleave L1's m-pairs between the L2/L3 m-blocks so
                # psA banks filled by L1 are never drained back-to-back by
                # its own (DVE-serial) relus
                l1 = s < nch
                h1c = h2c = h3c = None
                if l1:
                    prefetch(s + PRE)
                if l1:
                    h1c = do_L1(s, ms=range(0, 2))
                if 0 <= s - 1 < nch:
                    h2c = do_L23(s - 1, W2q, MC_B2, 2, ms=range(0, 4),
                                 alt=(s - 1 >= nch - 2))
                if l1:
                    do_L1(s, ms=range(2, 4), h1_cur=h1c)
                if 0 <= s - 1 < nch:
                    do_L23(s - 1, W2q, MC_B2, 2, ms=range(4, 8), h_cur=h2c,
                           alt=(s - 1 >= nch - 2))
                if l1:
                    do_L1(s, ms=range(4, 6), h1_cur=h1c)
                if 0 <= s - 2 < nch:
                    h3c = do_L23(s - 2, W3q, MC_B3, 3, ms=range(0, 4),
                                 alt=(s - 2 >= nch - 2))
                if l1:
                    do_L1(s, ms=range(6, 8), h1_cur=h1c)
                if 0 <= s - 2 < nch:
                    do_L23(s - 2, W3q, MC_B3, 3, ms=range(4, 8), h_cur=h3c,
                           alt=(s - 2 >= nch - 2))
                if 0 <= s - 3 < nch:
                    do_L4(s - 3, alt=(s - 3 >= nch - 2), fs=(0, 1))
                    flush_pool(s - 2)
                    do_L4(s - 3, alt=(s - 3 >= nch - 2), fs=(2, 3))
                else:
                    flush_pool(s - 2)
                if 0 <= s - 4 < nch:
                    do_scores(s - 4, s)
                if s >= nch:
                    # pipeline drain: no L1..L4 work left to hide behind,
                    # flush pooling as soon as the score chain lands
                    flush_pool(s)
            flush_pool(1 << 30)

            # ---- tail: final per-utterance MLP (f32r) ----
            # denom[s] = sum_p eacc[p, s] via one tiny PE matmul -> [S, 1]
            psd = psA.tile([S, 8], F32, tag="mm")
            nc.tensor.matmul(psd[:], eacc[:], ones_col,
                             start=True, stop=True)
            fc = colpool.tile([P, 16], F32, tag="col")
            nc.vector.tensor_copy(out=fc[:S, 0:1], in_=psd[:, 0:1])
            nc.vector.reciprocal(fc[:S, 1:2], fc[:S, 0:1])
            # denom as an f32r row [1, S] (rank-1 b6*denom matmul lhsT)
            psr = psA.tile([1, S], F32, tag="mm")
            nc.tensor.transpose(psr[:], fc[:S, 0:1], idents[0])
            drow = wpool.tile([1, S], F32R, tag="drow")
            nc.vector.tensor_copy(out=drow[:], in_=psr[:])

            # pooled PSUM -> SBUF (unscaled; normalization commutes out
            # of the relu -- r*relu(z + denom*b6) -- and is applied with
            # b7 in one fused op on the final [32, 10] tensor)
            pooled_sb = wpool.tile([S, HID], F32, tag="poolsb")
            nc.vector.tensor_copy(out=pooled_sb[:, :512], in_=pooled0[:])
            nc.vector.tensor_copy(out=pooled_sb[:, 512:], in_=pooled1[:])

            # transpose pooled -> bf16 pooledT [hin, seg]
            pooledT = wpool.tile([P, KS, S], BF16, tag="pooledT")
            gTb = wpool.tile([P, KS, S], BF16, tag="gTb")
            for k in range(KS):
                pst = psA.tile([P, S], F32, tag="mm")
                nc.tensor.transpose(
                    pst[:], pooled_sb[:, 128 * k:128 * (k + 1)], idents[0])
                nc.vector.tensor_copy(out=pooledT[:, k, :], in_=pst[:])

            # hout-major W6 stage: psgT_h [128, 32] = sum_k W6_k^T@pooledT_k
            # + b6[hout]*denom[seg] (rank-1), then relu -> bf16 gTb.
            # Small free dim (32) makes the bf16 matmuls ~4x cheaper than
            # the seg-major f32r orientation, and no gT transposes remain.
            for hh in range(KS):
                psg = psA.tile([P, S], F32, tag="mm")
                for k in range(KS):
                    nc.tensor.matmul(psg[:],
                                     W6s[k][:, 128 * hh:128 * (hh + 1)],
                                     pooledT[:, k, :],
                                     start=(k == 0), stop=False)
                nc.tensor.matmul(psg[:], b6s[:, 128 * hh:128 * (hh + 1)],
                                 drow[:], start=False, stop=True)
                if hh % 2 == 0:
                    nc.scalar.activation(gTb[:, hh, :], psg[:], RELU)
                else:
                    nc.vector.tensor_scalar_max(gTb[:, hh, :], psg[:], 0.0)

            # out = (gTb^T @ W7) * (1/denom) + b7  (bf16 matmuls, fused
            # scale+bias on DVE)
            pso = psA.tile([S, 16], F32, tag="mm")
            for k in range(KS):
                nc.tensor.matmul(pso[:, :NCLS], gTb[:, k, :], W7bv[:, k, :],
                                 start=(k == 0), stop=(k == KS - 1))
            oc = colpool.tile([S, 16], F32, tag="col")
            nc.vector.scalar_tensor_tensor(
                out=oc[:, :NCLS], in0=pso[:, :NCLS], scalar=fc[:S, 1:2],
                in1=misc[:S, MC_B7BC:MC_B7BC + NCLS],
                op0=MULT, op1=ADD)
            nc.sync.dma_start(out_d.ap()[:], oc[:, :NCLS])

    nc.compile()
    return nc


def _q8(a: np.ndarray) -> np.ndarray:
    return np.asarray(a, dtype=np.float32).astype(E4NP)


def _pack_dr(W: np.ndarray) -> np.ndarray:
    """[1024, N] weight matrix -> DoubleRow fp8 layout [KP, 128, 2, N]."""
    return np.ascontiguousarray(
        _q8(W).reshape(KP, 2, P, -1).transpose(0, 2, 1, 3))


def prepare_inputs(x, W1, b1, W2, b2, W3, b3, W4, b4, W5, b5, W6, b6, W7, b7,
                   lengths):
    """Host-side sharding/packing. Returns (in_maps, bins, m_pad)."""
    x = np.ascontiguousarray(np.asarray(x, dtype=np.float32))
    lengths = np.asarray(lengths)
    total = x.shape[0]
    seg_ids = _segment_ids(lengths, total)
    counts = np.bincount(seg_ids, minlength=NSEG).astype(np.int64)
    starts = np.zeros(NSEG + 1, dtype=np.int64)
    starts[1:] = np.cumsum(counts)

    bins = _balance_segments(counts)
    core_frames = [int(sum(counts[s] for s in b)) for b in bins]
    m_pad = ((max(core_frames) + CH - 1) // CH) * CH
    frt = m_pad // P

    W1p = np.zeros((P, HID), dtype=np.float32)
    W1p[:FEAT] = np.asarray(W1, dtype=np.float32)
    W1p[FEAT] = np.asarray(b1, dtype=np.float32)
    # DoubleRow over two 64-feature halves: [64, 2, HID]
    W1q = np.ascontiguousarray(
        _q8(W1p).reshape(2, P // 2, HID).transpose(1, 0, 2))

    misc = np.zeros((P, 256), dtype=np.float32)
    misc[:, MC_B2:MC_B2 + KS] = np.asarray(b2, np.float32).reshape(KS, P).T
    misc[:, MC_B3:MC_B3 + KS] = np.asarray(b3, np.float32).reshape(KS, P).T
    misc[:, MC_B5] = np.float32(np.asarray(b5, np.float32).reshape(-1)[0])
    misc[:SEGS_PER_CORE, MC_B7BC:MC_B7BC + NCLS] = np.asarray(
        b7, np.float32).reshape(1, NCLS)
    for q in range(4):
        misc[32 * q:32 * q + 32, MC_ID + 32 * q:MC_ID + 32 * q + 32] = np.eye(
            32, dtype=np.float32)

    mmcc = np.zeros((P, P), dtype=np.float32)
    mmcc[:, MM_ONES:MM_ONES + 8] = 1.0
    mmcc[:, MM_W7:MM_W7 + KS * NCLS] = np.asarray(W7, np.float32).reshape(
        KS, P, NCLS).transpose(1, 0, 2).reshape(P, KS * NCLS)

    rowm = np.zeros((1, 192), dtype=np.float32)
    rowm[0, RW_ONES:RW_ONES + P] = 1.0
    rowm[0, RW_B7:RW_B7 + NCLS] = np.asarray(b7, np.float32).reshape(-1)

    b4q = np.zeros((1, 2, HID), dtype=E4NP)
    b4q[0, 0, :] = _q8(np.asarray(b4, np.float32).reshape(-1))
    onesq = np.zeros((1, 2, P), dtype=E4NP)
    onesq[0, 0, :] = np.float32(1.0)

    shared = dict(
        W1q=W1q,
        W2q=_pack_dr(np.asarray(W2, np.float32)),
        W3q=_pack_dr(np.asarray(W3, np.float32)),
        W4q=_pack_dr(np.asarray(W4, np.float32)),
        W5rep=np.broadcast_to(
            np.asarray(W5, np.float32).reshape(1, HID).astype(
                ml_dtypes.bfloat16), (P, HID)).copy(),
        W6=np.ascontiguousarray(
            np.asarray(W6, np.float32).astype(ml_dtypes.bfloat16)),
        W7b=np.ascontiguousarray(
            np.asarray(W7, np.float32).reshape(KS, P, NCLS)
            .transpose(1, 0, 2).reshape(P, KS * NCLS)
            .astype(ml_dtypes.bfloat16)),
        b4q=b4q,
        onesq=onesq,
        b6r=np.asarray(b6, np.float32).reshape(1, HID),
        miscc=misc,
        mmcc=mmcc,
        rowm=rowm,
    )

    in_maps = []
    for core in range(NCORES):
        segs = bins[core]
        xs = [x[starts[s]:starts[s + 1]] for s in segs]
        xcat = np.concatenate(xs, axis=0) if xs else np.zeros((0, FEAT), np.float32)
        n = xcat.shape[0]
        xT = np.zeros((P, m_pad), dtype=np.float32)
        xT[:FEAT, :n] = xcat.T
        xT[FEAT, :n] = 1.0  # constant feature -> b1
        A = np.zeros((m_pad, SEGS_PER_CORE), dtype=np.float32)
        off = 0
        for j, s in enumerate(segs):
            ln = int(counts[s])
            A[off:off + ln, j] = 1.0
            off += ln
        im = dict(shared)
        # fp8 x, DoubleRow halves: xq[p, i, col] = xpad[i*64 + p, col]
        im["xq"] = np.ascontiguousarray(
            _q8(xT).reshape(2, P // 2, m_pad).transpose(1, 0, 2))
        # partition-major layout [P, frt, S]: Ah[p, t, s] = A[t*128 + p, s]
        im["Amat"] = np.ascontiguousarray(
            A.reshape(frt, P, SEGS_PER_CORE).transpose(1, 0, 2))
        in_maps.append(im)
    return in_maps, bins, m_pad


_PROGRAM_CACHE: dict[int, object] = {}


def kernel(**inputs) -> np.ndarray:
    in_maps, bins, m_pad = prepare_inputs(**inputs)
    nc = _PROGRAM_CACHE.get(m_pad)
    if nc is None:
        nc = _build_program(m_pad)
        _PROGRAM_CACHE[m_pad] = nc
    res = run_bass_kernel_spmd(nc, in_maps, core_ids=list(range(NCORES)))
    out = np.zeros((NSEG, NCLS), dtype=np.float32)
    for core in range(NCORES):
        out[bins[core]] = res.results[core]["out"]
    return out


# revision 85
# speedup vs baseline: 1.0165x; 1.0005x over previous
"""Trainium2 Bass kernel for nn_Dnn_with_Attention (ragged attention-pooled DNN).

Contract: kernel(**inputs) takes FULL unsharded numpy inputs (keys as in
reference.setup_inputs()) and returns the FULL [256, 10] float32 output.

Strategy (data-parallel over utterances, 8 NeuronCores):
  - Host: balance the 256 segments over 8 cores (32 whole segments each,
    greedy LPT + swap refinement -> every core at exactly sum/8 frames on
    the benchmark lengths, so m_pad = 16384 = 32 chunks), gather each
    core's frames, pack x as fp8 feature-major pairs [64, 2, M_PAD] and
    build a per-frame one-hot segment membership matrix A.  A row of ones
    is appended as feature 78 so b1 folds into W1.  All four weight
    matrices are pre-quantized to fp8 e4m3 in DoubleRow pair layout.
  - Device (per core): ALL four layers run as fp8(e4m3) DoubleRow matmuls
    (two 128-row contraction tiles per pass); inter-layer activations are
    stored e4m3, h4 in bf16.  b4 is added via a tiny fp8 ones-row
    DoubleRow matmul inside the same PSUM accumulation group.
  - A fully static (no hardware loop) skewed software pipeline over
    512-frame chunks: at step s the PE runs L2(s-1) and L3(s-2) in 4-wide
    m-blocks with L1(s)'s m-pairs INTERLEAVED between them (so psA banks
    filled by L1 are never drained back-to-back by its own DVE-serial
    relus), then L4(s-3) in two halves with the (deferred, s-6)
    pooling matmuls between them, and scores(s-4).  x/A are
    DMA-prefetched per chunk 3 steps ahead; W1/W2 stream on the scalar
    engine's DMA queue at startup, in parallel with chunk 0's inputs.
  - Activations: L2/L3 m<6 + all L4 relus on scalar (bias via the
    activation unit), L1 relus + L2/L3 m>=6 on DVE; the score reduction
    is a DVE 2x tensor_tensor multiply + 4x tensor_scalar accumulate;
    exp on scalar; clamp/mask/eacc on gpsimd.  During the pipeline drain
    the last chunks alternate act engines per m-tile (latency, not
    throughput, binds there) and the last chunk's h4 requant runs as
    scalar Copy activations.
  - Segment softmax pooling as fp8 DoubleRow PE matmuls contracting TWO
    128-frame tiles per pass (a DR pair (k, k+128) is the SAME partition
    of two consecutive frame-tiles, so E and a gpsimd-requantized fp8
    copy of h4 are stored as pair tiles [128, 2, *] with no partition
    shuffle), accumulated into two persistent PSUM banks (DR matmuls
    require dst partition 0), deferred two steps behind the score chain.
    The softmax denominator is an SBUF f32r accumulator (eacc += E on
    gpsimd, the same quantized values the numerator uses) reduced by one
    tiny matmul in the tail.
  - Tail: the final per-utterance MLP runs hout-major in bf16: after
    the pooled->pooledT transposes, psgT_h [128, 32] = sum_k W6_k^T @
    pooledT_k (+ b6[hout]*denom[seg] rank-1) so every matmul has free
    dim 32 (~4x cheaper than the seg-major f32r orientation) and the
    gT transposes disappear -- relu writes bf16 gTb tiles that feed the
    W7 matmuls directly.  The softmax normalization commutes out of the
    relu (r*relu(z + denom*b6)) and is applied together with b7 in one
    fused scalar_tensor_tensor on the final [32, 10] tensor.
"""

import sys

sys.path.insert(0, "/opt/trn_rl_repo")

import numpy as np
import ml_dtypes

import concourse.bass as bass
import concourse.mybir as mybir
import concourse.tile as tile
from concourse import bacc
from concourse.bass_utils import run_bass_kernel_spmd

P = 128
FEAT = 78
HID = 1024
NCLS = 10
NSEG = 256
NCORES = 8
SEGS_PER_CORE = NSEG // NCORES
CH = 512           # frames per chunk (free dim of layer-1..3 matmuls)
FRT_PER_CH = CH // P
NCOL = 256         # moving-dim columns per DoubleRow matmul (HW limit)
KS = HID // P      # 8 k-subtiles
KP = KS // 2       # 4 DoubleRow k-pairs
F32 = mybir.dt.float32
F32R = mybir.dt.float32r
F8 = mybir.dt.float8e4
BF16 = mybir.dt.bfloat16
E4NP = ml_dtypes.float8_e4m3

# misc constant tile column layout ([128, 256] f32, host-packed)
MC_B2 = 0          # cols 0..7   : b2 striped [128, 8]
MC_B3 = 8          # cols 8..15  : b3 striped
MC_B5 = 17         # col 17      : b5 replicated down partitions
MC_B7BC = 96       # cols 96..105, rows 0..31: b7 broadcast to 32 rows
MC_ID = 128        # cols 128..255: four 32x32 identity blocks, block q at
                   # rows 32q..32q+32, cols 128+32q..128+32q+32
# f32r matmul-constants tile ([128, 128])
MM_ONES = 0        # cols 0..7   : ones columns (denom matmul rhs, N=8)
MM_W7 = 16         # cols 16..95 : W7 as [128, 8, 10]
# row constants tile ([1, 192] f32r, host-packed)
RW_ONES = 0        # cols 0..127 : ones row
RW_B7 = 128        # cols 128..137 : b7


def _segment_ids(lengths: np.ndarray, total: int) -> np.ndarray:
    """Replicate jnp.repeat(arange(n), lengths, total_repeat_length=total)."""
    lengths = np.asarray(lengths, dtype=np.int64)
    seg = np.repeat(np.arange(lengths.shape[0], dtype=np.int32), np.maximum(lengths, 0))
    if seg.shape[0] >= total:
        return seg[:total]
    pad_val = seg[-1] if seg.shape[0] > 0 else np.int32(0)
    return np.concatenate([seg, np.full(total - seg.shape[0], pad_val, np.int32)])


def _balance_segments(lengths: np.ndarray) -> list[list[int]]:
    """Assign 256 segments to 8 cores, 32 each, minimizing max frame count.

    Greedy LPT init + single-move / pairwise-swap local search.  On the
    benchmark lengths this reaches a perfect partition (all cores at
    sum/8 frames), saving one 512-frame chunk of padding.
    """
    lengths = np.asarray(lengths, dtype=np.int64)
    cap = SEGS_PER_CORE
    order = np.argsort(-lengths, kind="stable")
    loads = [0] * NCORES
    bins: list[list[int]] = [[] for _ in range(NCORES)]
    for s in order:
        cands = [c for c in range(NCORES) if len(bins[c]) < cap]
        c = min(cands, key=lambda c: (loads[c], c))
        bins[c].append(int(s))
        loads[c] += int(lengths[s])
    target = (int(lengths.sum()) + NCORES - 1) // NCORES
    for _ in range(4096):
        hi = max(range(NCORES), key=lambda c: loads[c])
        if loads[hi] <= target:
            break
        best = None
        for other in range(NCORES):
            if other == hi:
                continue
            if len(bins[other]) < cap:
                for i, s in enumerate(bins[hi]):
                    d = int(lengths[s])
                    newmax = max(loads[hi] - d, loads[other] + d)
                    if best is None or newmax < best[0]:
                        best = (newmax, hi, other, i, None)
            for i, s1 in enumerate(bins[hi]):
                d1 = int(lengths[s1])
                for j, s2 in enumerate(bins[other]):
                    d2 = int(lengths[s2])
                    if d1 <= d2:
                        continue
                    newmax = max(loads[hi] - d1 + d2, loads[other] + d1 - d2)
                    if best is None or newmax < best[0]:
                        best = (newmax, hi, other, i, j)
        if best is None or best[0] >= loads[hi]:
            break
        _, a, b, i, j = best
        s1 = bins[a].pop(i)
        if j is None:
            bins[b].append(s1)
            loads[a] -= int(lengths[s1])
            loads[b] += int(lengths[s1])
        else:
            s2 = bins[b][j]
            bins[b][j] = s1
            bins[a].append(s2)
            loads[a] += int(lengths[s2]) - int(lengths[s1])
            loads[b] += int(lengths[s1]) - int(lengths[s2])
    for b in bins:
        b.sort()
    return bins


PRE = 3            # chunk DMA prefetch distance (steps)


def _build_program(m_pad: int):
    """Emit the fully static Bass/Tile program for one core (m_pad frames)."""
    nch = m_pad // CH
    frt = m_pad // P
    S = SEGS_PER_CORE

    nc = bacc.Bacc("TRN2", target_bir_lowering=False, debug=False,
                   num_devices=NCORES)

    xT_d = nc.dram_tensor("xq", [P // 2, 2, m_pad], F8, kind="ExternalInput")
    A_d = nc.dram_tensor("Amat", [P, frt, S], F32, kind="ExternalInput")
    W1_d = nc.dram_tensor("W1q", [P // 2, 2, HID], F8, kind="ExternalInput")
    W2_d = nc.dram_tensor("W2q", [KP, P, 2, HID], F8, kind="ExternalInput")
    W3_d = nc.dram_tensor("W3q", [KP, P, 2, HID], F8, kind="ExternalInput")
    W4_d = nc.dram_tensor("W4q", [KP, P, 2, HID], F8, kind="ExternalInput")
    W5_d = nc.dram_tensor("W5rep", [P, HID], BF16, kind="ExternalInput")
    W6_d = nc.dram_tensor("W6", [HID, HID], BF16, kind="ExternalInput")
    W7b_d = nc.dram_tensor("W7b", [P, KS * NCLS], BF16, kind="ExternalInput")
    b4_d = nc.dram_tensor("b4q", [1, 2, HID], F8, kind="ExternalInput")
    on_d = nc.dram_tensor("onesq", [1, 2, P], F8, kind="ExternalInput")
    b6_d = nc.dram_tensor("b6r", [1, HID], F32R, kind="ExternalInput")
    misc_d = nc.dram_tensor("miscc", [P, 256], F32, kind="ExternalInput")
    mmc_d = nc.dram_tensor("mmcc", [P, P], F32R, kind="ExternalInput")
    row_d = nc.dram_tensor("rowm", [1, 192], F32R, kind="ExternalInput")
    out_d = nc.dram_tensor("out", [S, NCLS], F32, kind="ExternalOutput")

    RELU = mybir.ActivationFunctionType.Relu
    EXP = mybir.ActivationFunctionType.Exp
    MULT = mybir.AluOpType.mult
    ADD = mybir.AluOpType.add
    MAX = mybir.AluOpType.max
    DR = mybir.MatmulPerfMode.DoubleRow

    with tile.TileContext(nc) as tc:
        with (
            tc.tile_pool(name="wpool", bufs=1) as wpool,
            tc.tile_pool(name="xpool", bufs=4) as xpool,
            tc.tile_pool(name="apool", bufs=8) as apool,
            tc.tile_pool(name="hpool", bufs=2) as hpool,
            tc.tile_pool(name="h1pool", bufs=3) as h1pool,
            tc.tile_pool(name="h4pool", bufs=18) as h4pool,
            tc.tile_pool(name="spool", bufs=2) as spool,
            tc.tile_pool(name="colpool", bufs=8) as colpool,
            tc.tile_pool(name="epool", bufs=12) as epool,
            tc.tile_pool(name="psA", bufs=4, space="PSUM") as psA,
            tc.tile_pool(name="psB", bufs=2, space="PSUM") as psB,
            tc.tile_pool(name="psAcc", bufs=1, space="PSUM") as psAcc,
        ):
            # ---- chunk input prefetch (per-chunk x/A slices) ----
            xs: dict[int, object] = {}
            As: dict[int, object] = {}

            def prefetch(c):
                if not (0 <= c < nch) or c in xs:
                    return
                xt = xpool.tile([P // 2, 2, CH], F8, tag="x")
                nc.sync.dma_start(xt[:], xT_d.ap()[:, :, c * CH:(c + 1) * CH])
                at = apool.tile([P, FRT_PER_CH, S], F32, tag="A")
                nc.sync.dma_start(
                    at[:], A_d.ap()[:, c * FRT_PER_CH:(c + 1) * FRT_PER_CH, :])
                xs[c] = xt
                As[c] = at

            # ---- resident constants/weights.  DMA issue order tracks first
            # use: W1 + chunk 0/1 first, then each layer's weights in the
            # order the pipeline-fill steps consume them; tail-only
            # constants (mmc/rowm/b6/W6) last.
            W1s = wpool.tile([P // 2, 2, HID], F8, tag="W1")
            # startup: W1 on the (idle) scalar queue so it runs in
            # parallel with chunk 0/1's x/A DMAs on the SP queue
            nc.gpsimd.dma_start(W1s[:, :, :128], W1_d.ap()[:, :, :128])
            nc.gpsimd.dma_start(W1s[:, :, 128:], W1_d.ap()[:, :, 128:])
            prefetch(0)
            prefetch(1)
            W2q = [wpool.tile([P, 2, HID], F8, tag=f"W2q{j}", name=f"W2q{j}")
                   for j in range(KP)]
            W3q = [wpool.tile([P, 2, HID], F8, tag=f"W3q{j}", name=f"W3q{j}")
                   for j in range(KP)]
            W4q = [wpool.tile([P, 2, HID], F8, tag=f"W4q{j}", name=f"W4q{j}")
                   for j in range(KP)]
            for j in range(KP):
                nc.scalar.dma_start(W2q[j][:], W2_d.ap()[j])
            misc = wpool.tile([P, 256], F32, tag="misc")
            nc.sync.dma_start(misc[:], misc_d.ap())
            prefetch(2)
            for j in range(KP):
                nc.sync.dma_start(W3q[j][:], W3_d.ap()[j])
            b4qs = wpool.tile([1, 2, HID], F8, tag="b4q")
            nc.sync.dma_start(b4qs[:], b4_d.ap())
            onesq = wpool.tile([1, 2, P], F8, tag="onesq")
            nc.sync.dma_start(onesq[:], on_d.ap())
            for j in range(KP):
                nc.sync.dma_start(W4q[j][:], W4_d.ap()[j])
            W5s = wpool.tile([P, HID], BF16, tag="W5")
            nc.sync.dma_start(W5s[:], W5_d.ap())
            mmc = wpool.tile([P, P], F32R, tag="mmc")
            nc.sync.dma_start(mmc[:], mmc_d.ap())
            rowm = wpool.tile([1, 192], F32R, tag="rowm")
            nc.sync.dma_start(rowm[:], row_d.ap())
            b6s = wpool.tile([1, HID], F32R, tag="b6")
            nc.sync.dma_start(b6s[:], b6_d.ap())
            W6s = []
            for k in range(KS):
                t = wpool.tile([P, HID], BF16, tag=f"W6k{k}")
                nc.sync.dma_start(t[:], W6_d.ap()[k * P:(k + 1) * P, :])
                W6s.append(t)
            W7b = wpool.tile([P, KS * NCLS], BF16, tag="W7b")
            nc.sync.dma_start(W7b[:], W7b_d.ap())
            W7bv = W7b.rearrange("p (o c) -> p o c", c=NCLS)

            ones_col = mmc[:, MM_ONES:MM_ONES + 8]
            b5col = misc[:, MC_B5:MC_B5 + 1]
            idents = [misc[32 * q:32 * q + 32, MC_ID + 32 * q:MC_ID + 32 * q + 32]
                      for q in range(4)]
            W7v = mmc[:, MM_W7:MM_W7 + KS * NCLS].rearrange(
                "p (o c) -> p o c", c=NCLS)
            b7row = rowm[:, RW_B7:RW_B7 + NCLS]
            ones_row = rowm[:, RW_ONES:RW_ONES + P]

            # persistent PSUM pooled accumulators, one bank per hidden
            # half at partitions 0..31 (DoubleRow matmuls require dst
            # partition 0)
            pooled0 = psAcc.tile([S, 512], F32, tag="pooled0")
            pooled1 = psAcc.tile([S, 512], F32, tag="pooled1")
            # softmax denominator accumulator (f32r: feeds the tail matmul)
            eacc = wpool.tile([P, S], F32R, tag="eacc")

            # ---- per-chunk layer emitters (skewed pipeline below) ----
            h1s: dict[int, object] = {}
            h2s: dict[int, object] = {}
            h3s: dict[int, object] = {}
            h4s: dict[int, list] = {}
            pend: list[tuple] = []

            def do_L1(u, n_scalar_relu=0, ms=None, h1_cur=None):
                # fp8 DoubleRow over two 64-feature halves (b1 folded via
                # ones feature); relu on DVE (scalar split during pipeline
                # fill when the scalar engine is otherwise idle).  h1 is
                # stored as four per-DR-pair tiles [P, 2, CH] so each of
                # L2's j-matmuls depends only on the two relu slices it
                # actually reads.
                if ms is None:
                    ms = range(KS)
                xg = xs[u]
                if h1_cur is None:
                    h1_cur = [h1pool.tile([P, 2, CH], F8, tag=f"h1p{j}",
                                          name=f"h1p{j}") for j in range(KP)]
                h1 = h1_cur
                for m in ms:
                    ps = psA.tile([P, CH], F32, tag="mm")
                    for c2 in range(CH // NCOL):
                        xt = xg[:, :, c2 * NCOL:(c2 + 1) * NCOL]
                        nc.tensor.matmul(ps[:, c2 * NCOL:(c2 + 1) * NCOL],
                                         W1s[:, :, m * P:(m + 1) * P],
                                         xt, start=True, stop=True,
                                         perf_mode=DR)
                    if m < n_scalar_relu:
                        nc.scalar.activation(h1[m // 2][:, m % 2, :], ps[:],
                                             RELU)
                    else:
                        nc.vector.tensor_scalar_max(h1[m // 2][:, m % 2, :],
                                                    ps[:], 0.0)
                h1s[u] = h1
                if max(ms) == KS - 1:
                    xs.pop(u)
                return h1

            def do_L23(u, Wq, boff, li, ms=None, h_cur=None, alt=False):
                # fp8 DoubleRow; relu+bias on scalar (m<6) / DVE (m>=6)
                if ms is None:
                    ms = range(KS)
                h_in = h1s[u] if li == 2 else h2s[u]
                tag = "h2p" if li == 2 else "h3p"
                if h_cur is None:
                    h_cur = [hpool.tile([P, 2, CH], F8, tag=f"{tag}{j}",
                                        name=f"{tag}{j}") for j in range(KP)]
                h_out = h_cur
                for m in ms:
                    ps = psA.tile([P, CH], F32, tag="mm")
                    for c2 in range(CH // NCOL):
                        seg = ps[:, c2 * NCOL:(c2 + 1) * NCOL]
                        for j in range(KP):
                            nc.tensor.matmul(
                                seg, Wq[j][:, :, m * P:(m + 1) * P],
                                h_in[j][:, :, c2 * NCOL:(c2 + 1) * NCOL],
                                start=(j == 0), stop=(j == KP - 1),
                                perf_mode=DR)
                    bcol = misc[:, boff + m:boff + m + 1]
                    if (m % 2 == 1) if alt else (m >= 6):
                        nc.vector.tensor_scalar(
                            out=h_out[m // 2][:, m % 2, :], in0=ps[:],
                            scalar1=bcol, scalar2=0.0,
                            op0=ADD, op1=MAX)
                    else:
                        nc.scalar.activation(
                            h_out[m // 2][:, m % 2, :], ps[:], RELU,
                            bias=bcol)
                (h2s if li == 2 else h3s)[u] = h_out
                if max(ms) == KS - 1:
                    (h1s if li == 2 else h2s).pop(u)
                return h_out

            def do_L4(u, alt=False, fs=None):
                # frame-major fp8 DR; relu on scalar; h4 bf16 (score DVE 2x
                # rate + 1 cycle/row pooling matmuls)
                if fs is None:
                    fs = range(FRT_PER_CH)
                h3 = h3s[u]
                tiles = h4s.get(u, [])
                for f in fs:
                    h4 = h4pool.tile([P, HID], BF16, tag="h4")
                    for n in range(2):
                        ps4 = psB.tile([P, 512], F32, tag="l4")
                        for c2 in range(2):
                            seg = ps4[:, c2 * 256:(c2 + 1) * 256]
                            col0 = n * 512 + c2 * 256
                            for j in range(KP):
                                nc.tensor.matmul(
                                    seg,
                                    h3[j][:, :, f * P:(f + 1) * P],
                                    W4q[j][:, :, col0:col0 + 256],
                                    start=(j == 0), stop=False,
                                    perf_mode=DR)
                            nc.tensor.matmul(
                                seg, onesq[:],
                                b4qs[:, :, col0:col0 + 256],
                                start=False, stop=True, perf_mode=DR)
                        if alt and n == 1:
                            nc.vector.tensor_scalar_max(
                                h4[:, n * 512:(n + 1) * 512], ps4[:], 0.0)
                        else:
                            nc.scalar.activation(
                                h4[:, n * 512:(n + 1) * 512], ps4[:], RELU)
                    tiles.append(h4)
                h4s[u] = tiles
                if max(fs) == FRT_PER_CH - 1:
                    h3s.pop(u)

            def do_scores(u, step):
                # d = sum(h4*W5rep) via DVE mult (2x) + reduce (4x);
                # e = max(exp(d + b5), 1) on scalar/gpsimd; E = A*e and
                # eacc += E on gpsimd.  For the fp8 DoubleRow pooling, h4
                # is also requantized to fp8 PAIR tiles [128, 2, 1024]
                # (slot i = frame-tile 2p+i) on gpsimd — emitted FIRST so
                # the (long-ready) copies fill the Pool queue's idle front
                # while the et chain waits for the exp results.
                ag = As.pop(u)
                first_u = (u == 0)
                tiles = h4s.pop(u)
                h4q = []
                CPY = mybir.ActivationFunctionType.Copy
                for pi in range(FRT_PER_CH // 2):
                    hq = h4pool.tile([P, 2, HID], F8, tag="h4q")
                    if u >= nch - 1:
                        # drain: scalar engine is idle; freeing the Pool
                        # queue lets the final et chain -> pooling finish
                        # sooner
                        nc.scalar.activation(hq[:, 0, :], tiles[2 * pi][:],
                                             CPY)
                        nc.scalar.activation(hq[:, 1, :],
                                             tiles[2 * pi + 1][:], CPY)
                    else:
                        nc.gpsimd.tensor_scalar_add(hq[:, 0, :],
                                                    tiles[2 * pi][:], 0.0)
                        nc.gpsimd.tensor_scalar_add(hq[:, 1, :],
                                                    tiles[2 * pi + 1][:], 0.0)
                    h4q.append(hq)
                etp = None
                for f, h4 in enumerate(tiles):
                    prod = spool.tile([P, HID], BF16, tag="sc")
                    ct = colpool.tile([P, 16], F32, tag="col")
                    nc.vector.tensor_tensor(
                        out=prod[:], in0=h4[:], in1=W5s[:], op=MULT)
                    nc.vector.tensor_scalar(
                        out=prod[:], in0=prod[:], scalar1=1.0,
                        scalar2=0.0, op0=MULT, op1=ADD,
                        accum_out=ct[:, 0:1])
                    nc.scalar.activation(ct[:, 1:2], ct[:, 0:1], EXP,
                                         bias=b5col)
                    nc.gpsimd.tensor_scalar_max(ct[:, 2:3], ct[:, 1:2], 1.0)
                    if f % 2 == 0:
                        etp = epool.tile([P, 2, S], F8, tag="E")
                    nc.gpsimd.tensor_scalar_mul(
                        etp[:, f % 2, :], ag[:, f, :], ct[:, 2:3])
                    # eacc accumulates the SAME fp8-quantized E values the
                    # pooling numerator uses, so softmax weights sum to 1
                    if first_u and f == 0:
                        nc.gpsimd.tensor_scalar_add(eacc[:],
                                                    etp[:, 0, :], 0.0)
                    else:
                        nc.gpsimd.tensor_tensor(
                            out=eacc[:], in0=eacc.bitcast(F32)[:],
                            in1=etp[:, f % 2, :], op=ADD)
                    if f % 2 == 1:
                        st = bool(first_u and f == 1)
                        sp = bool(u == nch - 1 and f == FRT_PER_CH - 1)
                        pend.append((etp, h4q[f // 2], st, sp, step))

            def flush_pool(max_step, limit=1 << 30):
                # fp8 DoubleRow pooling matmuls (K=256: two frame-tiles per
                # pass) for score chains born at step <= max_step: 4
                # col-group quarter matmuls into the single pooled bank
                while pend and pend[0][4] <= max_step and limit > 0:
                    limit -= 1
                    etp, hq, st, sp, _ = pend.pop(0)
                    for h, pl in enumerate((pooled0, pooled1)):
                        # one full-row (N=512) fp8 DoubleRow matmul per
                        # hidden half (contraction = two frame-tiles)
                        nc.tensor.matmul(
                            pl[:], etp[:],
                            hq[:, :, 512 * h:512 * (h + 1)],
                            start=st, stop=sp, perf_mode=DR)

            # ---- main static pipeline over chunks ----
            # 4-deep skewed pipeline; chunk 0's L1 is pre-emitted with its
            # relus split across the still-idle scalar engine so step 1's
            # L2(0) never races the DVE queue.
            if nch >= 1:
                do_L1(0, n_scalar_relu=4)
            prefetch(3)
            for s in range(1, nch + 5):
                # interleave L1's m-pairs between the L2/L3 m-blocks so
                # psA banks filled by L1 are never drained back-to-back by
                # its own (DVE-serial) relus
                l1 = s < nch
                h1c = h2c = h3c = None
                if l1:
                    prefetch(s + PRE)
                if l1:
                    h1c = do_L1(s, ms=range(0, 2))
                if 0 <= s - 1 < nch:
                    h2c = do_L23(s - 1, W2q, MC_B2, 2, ms=range(0, 4),
                                 alt=(s - 1 >= nch - 2))
                if l1:
                    do_L1(s, ms=range(2, 4), h1_cur=h1c)
                if 0 <= s - 1 < nch:
                    do_L23(s - 1, W2q, MC_B2, 2, ms=range(4, 8), h_cur=h2c,
                           alt=(s - 1 >= nch - 2))
                if l1:
                    do_L1(s, ms=range(4, 6), h1_cur=h1c)
                if 0 <= s - 2 < nch:
                    h3c = do_L23(s - 2, W3q, MC_B3, 3, ms=range(0, 4),
                                 alt=(s - 2 >= nch - 2))
                if l1:
                    do_L1(s, ms=range(6, 8), h1_cur=h1c)
                if 0 <= s - 2 < nch:
                    do_L23(s - 2, W3q, MC_B3, 3, ms=range(4, 8), h_cur=h3c,
                           alt=(s - 2 >= nch - 2))
                if 0 <= s - 3 < nch:
                    do_L4(s - 3, alt=(s - 3 >= nch - 2), fs=(0, 1))
                    flush_pool(s - 2)
                    do_L4(s - 3, alt=(s - 3 >= nch - 2), fs=(2, 3))
                else:
                    flush_pool(s - 2)
                if 0 <= s - 4 < nch:
                    do_scores(s - 4, s)
                if s >= nch:
                    # pipeline drain: no L1..L4 work left to hide behind,
                    # flush pooling as soon as the score chain lands
                    flush_pool(s)
            flush_pool(1 << 30)

            # ---- tail: final per-utterance MLP (f32r) ----
            # denom[s] = sum_p eacc[p, s] via one tiny PE matmul -> [S, 1]
            psd = psA.tile([S, 8], F32, tag="mm")
            nc.tensor.matmul(psd[:], eacc[:], ones_col,
                             start=True, stop=True)
            fc = colpool.tile([P, 16], F32, tag="col")
            nc.vector.tensor_copy(out=fc[:S, 0:1], in_=psd[:, 0:1])
            nc.vector.reciprocal(fc[:S, 1:2], fc[:S, 0:1])
            # denom as an f32r row [1, S] (rank-1 b6*denom matmul lhsT)
            psr = psA.tile([1, S], F32, tag="mm")
            nc.tensor.transpose(psr[:], fc[:S, 0:1], idents[0])
            drow = wpool.tile([1, S], F32R, tag="drow")
            nc.vector.tensor_copy(out=drow[:], in_=psr[:])

            # pooled PSUM -> SBUF (unscaled; normalization commutes out
            # of the relu -- r*relu(z + denom*b6) -- and is applied with
            # b7 in one fused op on the final [32, 10] tensor)
            pooled_sb = wpool.tile([S, HID], F32, tag="poolsb")
            nc.vector.tensor_copy(out=pooled_sb[:, :512], in_=pooled0[:])
            nc.vector.tensor_copy(out=pooled_sb[:, 512:], in_=pooled1[:])

            # transpose pooled -> bf16 pooledT [hin, seg]
            pooledT = wpool.tile([P, KS, S], BF16, tag="pooledT")
            gTb = wpool.tile([P, KS, S], BF16, tag="gTb")
            psg0t = psB.tile([P, S], F32, tag="l4")
            for k in range(KS):
                pst = psA.tile([P, S], F32, tag="mm")
                nc.tensor.transpose(
                    pst[:], pooled_sb[:, 128 * k:128 * (k + 1)], idents[0])
                nc.vector.tensor_copy(out=pooledT[:, k, :], in_=pst[:])
                # pipeline the first hout group's k-matmul right behind
                # the transpose that produces its operand
                nc.tensor.matmul(psg0t[:], W6s[k][:, 0:128],
                                 pooledT[:, k, :],
                                 start=(k == 0), stop=False)

            # hout-major W6 stage: psgT_h [128, 32] = sum_k W6_k^T@pooledT_k
            # + b6[hout]*denom[seg] (rank-1), then relu -> bf16 gTb.
            # Small free dim (32) makes the bf16 matmuls ~4x cheaper than
            # the seg-major f32r orientation, and no gT transposes remain.
            for hh in range(KS):
                if hh == 0:
                    psg = psg0t
                else:
                    psg = psA.tile([P, S], F32, tag="mm")
                    for k in range(KS):
                        nc.tensor.matmul(psg[:],
                                         W6s[k][:, 128 * hh:128 * (hh + 1)],
                                         pooledT[:, k, :],
                                         start=(k == 0), stop=False)
                nc.tensor.matmul(psg[:], b6s[:, 128 * hh:128 * (hh + 1)],
                                 drow[:], start=False, stop=True)
                if hh % 2 == 0:
                    nc.scalar.activation(gTb[:, hh, :], psg[:], RELU)
                else:
                    nc.vector.tensor_scalar_max(gTb[:, hh, :], psg[:], 0.0)

            # out = (gTb^T @ W7) * (1/denom) + b7  (bf16 matmuls, fused
            # scale+bias on DVE)
            pso = psA.tile([S, 16], F32, tag="mm")
            for k in range(KS):
                nc.tensor.matmul(pso[:, :NCLS], gTb[:, k, :], W7bv[:, k, :],
                                 start=(k == 0), stop=(k == KS - 1))
            oc = colpool.tile([S, 16], F32, tag="col")
            nc.vector.scalar_tensor_tensor(
                out=oc[:, :NCLS], in0=pso[:, :NCLS], scalar=fc[:S, 1:2],
                in1=misc[:S, MC_B7BC:MC_B7BC + NCLS],
                op0=MULT, op1=ADD)
            nc.sync.dma_start(out_d.ap()[:], oc[:, :NCLS])

    nc.compile()
    return nc


def _q8(a: np.ndarray) -> np.ndarray:
    return np.asarray(a, dtype=np.float32).astype(E4NP)


def _pack_dr(W: np.ndarray) -> np.ndarray:
    """[1024, N] weight matrix -> DoubleRow fp8 layout [KP, 128, 2, N]."""
    return np.ascontiguousarray(
        _q8(W).reshape(KP, 2, P, -1).transpose(0, 2, 1, 3))


def prepare_inputs(x, W1, b1, W2, b2, W3, b3, W4, b4, W5, b5, W6, b6, W7, b7,
                   lengths):
    """Host-side sharding/packing. Returns (in_maps, bins, m_pad)."""
    x = np.ascontiguousarray(np.asarray(x, dtype=np.float32))
    lengths = np.asarray(lengths)
    total = x.shape[0]
    seg_ids = _segment_ids(lengths, total)
    counts = np.bincount(seg_ids, minlength=NSEG).astype(np.int64)
    starts = np.zeros(NSEG + 1, dtype=np.int64)
    starts[1:] = np.cumsum(counts)

    bins = _balance_segments(counts)
    core_frames = [int(sum(counts[s] for s in b)) for b in bins]
    m_pad = ((max(core_frames) + CH - 1) // CH) * CH
    frt = m_pad // P

    W1p = np.zeros((P, HID), dtype=np.float32)
    W1p[:FEAT] = np.asarray(W1, dtype=np.float32)
    W1p[FEAT] = np.asarray(b1, dtype=np.float32)
    # DoubleRow over two 64-feature halves: [64, 2, HID]
    W1q = np.ascontiguousarray(
        _q8(W1p).reshape(2, P // 2, HID).transpose(1, 0, 2))

    misc = np.zeros((P, 256), dtype=np.float32)
    misc[:, MC_B2:MC_B2 + KS] = np.asarray(b2, np.float32).reshape(KS, P).T
    misc[:, MC_B3:MC_B3 + KS] = np.asarray(b3, np.float32).reshape(KS, P).T
    misc[:, MC_B5] = np.float32(np.asarray(b5, np.float32).reshape(-1)[0])
    misc[:SEGS_PER_CORE, MC_B7BC:MC_B7BC + NCLS] = np.asarray(
        b7, np.float32).reshape(1, NCLS)
    for q in range(4):
        misc[32 * q:32 * q + 32, MC_ID + 32 * q:MC_ID + 32 * q + 32] = np.eye(
            32, dtype=np.float32)

    mmcc = np.zeros((P, P), dtype=np.float32)
    mmcc[:, MM_ONES:MM_ONES + 8] = 1.0
    mmcc[:, MM_W7:MM_W7 + KS * NCLS] = np.asarray(W7, np.float32).reshape(
        KS, P, NCLS).transpose(1, 0, 2).reshape(P, KS * NCLS)

    rowm = np.zeros((1, 192), dtype=np.float32)
    rowm[0, RW_ONES:RW_ONES + P] = 1.0
    rowm[0, RW_B7:RW_B7 + NCLS] = np.asarray(b7, np.float32).reshape(-1)

    b4q = np.zeros((1, 2, HID), dtype=E4NP)
    b4q[0, 0, :] = _q8(np.asarray(b4, np.float32).reshape(-1))
    onesq = np.zeros((1, 2, P), dtype=E4NP)
    onesq[0, 0, :] = np.float32(1.0)

    shared = dict(
        W1q=W1q,
        W2q=_pack_dr(np.asarray(W2, np.float32)),
        W3q=_pack_dr(np.asarray(W3, np.float32)),
        W4q=_pack_dr(np.asarray(W4, np.float32)),
        W5rep=np.broadcast_to(
            np.asarray(W5, np.float32).reshape(1, HID).astype(
                ml_dtypes.bfloat16), (P, HID)).copy(),
        W6=np.ascontiguousarray(
            np.asarray(W6, np.float32).astype(ml_dtypes.bfloat16)),
        W7b=np.ascontiguousarray(
            np.asarray(W7, np.float32).reshape(KS, P, NCLS)
            .transpose(1, 0, 2).reshape(P, KS * NCLS)
            .astype(ml_dtypes.bfloat16)),
        b4q=b4q,
        onesq=onesq,
        b6r=np.asarray(b6, np.float32).reshape(1, HID),
        miscc=misc,
        mmcc=mmcc,
        rowm=rowm,
    )

    in_maps = []
    for core in range(NCORES):
        segs = bins[core]
        xs = [x[starts[s]:starts[s + 1]] for s in segs]
        xcat = np.concatenate(xs, axis=0) if xs else np.zeros((0, FEAT), np.float32)
        n = xcat.shape[0]
        xT = np.zeros((P, m_pad), dtype=np.float32)
        xT[:FEAT, :n] = xcat.T
        xT[FEAT, :n] = 1.0  # constant feature -> b1
        A = np.zeros((m_pad, SEGS_PER_CORE), dtype=np.float32)
        off = 0
        for j, s in enumerate(segs):
            ln = int(counts[s])
            A[off:off + ln, j] = 1.0
            off += ln
        im = dict(shared)
        # fp8 x, DoubleRow halves: xq[p, i, col] = xpad[i*64 + p, col]
        im["xq"] = np.ascontiguousarray(
            _q8(xT).reshape(2, P // 2, m_pad).transpose(1, 0, 2))
        # partition-major layout [P, frt, S]: Ah[p, t, s] = A[t*128 + p, s]
        im["Amat"] = np.ascontiguousarray(
            A.reshape(frt, P, SEGS_PER_CORE).transpose(1, 0, 2))
        in_maps.append(im)
    return in_maps, bins, m_pad


_PROGRAM_CACHE: dict[int, object] = {}


def kernel(**inputs) -> np.ndarray:
    in_maps, bins, m_pad = prepare_inputs(**inputs)
    nc = _PROGRAM_CACHE.get(m_pad)
    if nc is None:
        nc = _build_program(m_pad)
        _PROGRAM_CACHE[m_pad] = nc
    res = run_bass_kernel_spmd(nc, in_maps, core_ids=list(range(NCORES)))
    out = np.zeros((NSEG, NCLS), dtype=np.float32)
    for core in range(NCORES):
        out[bins[core]] = res.results[core]["out"]
    return out
